# revision 15
# baseline (speedup 1.0000x reference)
"""Trainium2 kernel for nn_BilevelFramework (path-based traffic assignment).

The oracle's inputs enumerate ALL simple paths of <=3 edges of the directed
graph encoded by ``time_mat`` (edge exists iff time > 0), grouped per OD pair
(group = o*N + d), padded to P_MAX with a dummy segment. For such inputs the
per-OD softmax over paths and the edge scatter-add collapse exactly into
dense 110x110 matrix algebra over W = exp(-lambda*time) .* (time>0):

  denom       = W + W^2 + W^3 - W.*(r(+)r) + W.*W.*W^T          (r = diag(W^2))
  C           = D / denom,  D = ode .* (ode>0) .* offdiag
  flows = W .* ( C + C W^T + W^T C + C (W^2)^T + W^T C W^T + (W^2)^T C
                 - W^T.*((h+g)(+)(h+g)) - C.*(r(+)r)
                 + 2 W.*W^T.*C + (W^T.*W^T).*C^T )
  with h = rowsum(W.*C), g = colsum(W.*C).

(The inclusion-exclusion terms remove non-simple paths, exactly matching the
reference's path enumeration constraints; verified to ~1e-15 rel in float64
and ~1e-6 in float32 against the oracle.)

The kernel verifies on the host that the path inputs are exactly that
enumeration (order-independent multiset check). If they are, it runs the
dense computation on the TRN2 NeuronCores via a Bass/Tile kernel (SPMD on
cores 0-7). Otherwise it falls back to a faithful elementwise computation.
"""

import os
import sys

import numpy as np

N = 110
P_MAX = 400000
NSEG = N * N + 1

LAST_EXEC_NS = None  # filled when KERNEL_TRACE=1


# --------------------------------------------------------------------------
# Host-side structure check: inputs == full <=3-edge simple-path enumeration?
# --------------------------------------------------------------------------

def _enumerate_records(A):
    """Record table [P, 12] (u0..2, v0..2, m0..2, oo, dd, g) of the full
    <=3-edge simple-path enumeration of adjacency A, or None if it would
    overflow P_MAX (the reference would truncate, which we don't model)."""
    idx = np.arange(N, dtype=np.int32)

    o1, d1 = np.nonzero(A)
    o1 = o1.astype(np.int32)
    d1 = d1.astype(np.int32)

    B2 = A[:, :, None] & A[None, :, :]
    B2 &= idx[:, None, None] != idx[None, None, :]  # o != d
    o2, k2, d2 = [x.astype(np.int32) for x in np.nonzero(B2)]

    B3 = (A[:, :, None, None] & A[None, :, :, None]) & A[None, None, :, :]
    B3 &= idx[:, None, None, None] != idx[None, None, :, None]  # o != b
    B3 &= idx[None, :, None, None] != idx[None, None, None, :]  # a != d
    B3 &= idx[:, None, None, None] != idx[None, None, None, :]  # o != d
    o3, a3, b3, d3 = [x.astype(np.int32) for x in np.nonzero(B3)]

    n1, n2, n3 = len(o1), len(o2), len(o3)
    total = n1 + n2 + n3
    if total > P_MAX:
        return None

    rec = np.zeros((P_MAX, 12), np.int32)
    rec[:, 11] = N * N  # padding group
    ofs = 0
    # 1-edge
    rec[ofs:ofs + n1, 0] = o1
    rec[ofs:ofs + n1, 3] = d1
    rec[ofs:ofs + n1, 6] = 1
    rec[ofs:ofs + n1, 9] = o1
    rec[ofs:ofs + n1, 10] = d1
    rec[ofs:ofs + n1, 11] = o1 * N + d1
    ofs += n1
    # 2-edge
    rec[ofs:ofs + n2, 0] = o2
    rec[ofs:ofs + n2, 1] = k2
    rec[ofs:ofs + n2, 3] = k2
    rec[ofs:ofs + n2, 4] = d2
    rec[ofs:ofs + n2, 6] = 1
    rec[ofs:ofs + n2, 7] = 1
    rec[ofs:ofs + n2, 9] = o2
    rec[ofs:ofs + n2, 10] = d2
    rec[ofs:ofs + n2, 11] = o2 * N + d2
    ofs += n2
    # 3-edge
    rec[ofs:ofs + n3, 0] = o3
    rec[ofs:ofs + n3, 1] = a3
    rec[ofs:ofs + n3, 2] = b3
    rec[ofs:ofs + n3, 3] = a3
    rec[ofs:ofs + n3, 4] = b3
    rec[ofs:ofs + n3, 5] = d3
    rec[ofs:ofs + n3, 6:9] = 1
    rec[ofs:ofs + n3, 9] = o3
    rec[ofs:ofs + n3, 10] = d3
    rec[ofs:ofs + n3, 11] = o3 * N + d3
    return rec


def _sort_rows(rec):
    # lexsort by all 12 columns (column 0 = most significant; any fixed
    # total order works for multiset comparison)
    order = np.lexsort(tuple(rec[:, c] for c in range(11, -1, -1)))
    return rec[order]


def _inputs_conform(time_mat, path_u, path_v, edge_mask, od_o, od_d, group):
    if (path_u.shape != (P_MAX, 3) or path_v.shape != (P_MAX, 3)
            or edge_mask.shape != (P_MAX, 3) or od_o.shape != (P_MAX,)
            or od_d.shape != (P_MAX,) or group.shape != (P_MAX,)
            or time_mat.shape != (N, N)):
        return False
    if np.any(np.diag(time_mat) != 0.0):
        return False
    if np.any(time_mat < 0.0):
        return False
    A = time_mat > 0.0
    rec = _enumerate_records(A)
    if rec is None:
        return False
    given = np.zeros((P_MAX, 12), np.int32)
    given[:, 0:3] = path_u
    given[:, 3:6] = path_v
    given[:, 6:9] = edge_mask
    given[:, 9] = od_o
    given[:, 10] = od_d
    given[:, 11] = group
    return bool(np.array_equal(_sort_rows(rec), _sort_rows(given)))


# --------------------------------------------------------------------------
# Dense Bass/Tile device kernel
# --------------------------------------------------------------------------

def _ensure_repo_on_path():
    try:
        import concourse  # noqa: F401
    except ImportError:
        for p in ("/opt/trn_rl_repo", os.path.expanduser("~/trn_rl_repo")):
            if os.path.isdir(p):
                sys.path.insert(0, p)
                break


_NC_CACHE = {}


def _build_dense_nc_raw(lam):
    """Hand-scheduled (raw bacc) dense-flows program.

    Same math as _build_dense_nc, but explicit per-engine streams and
    counting semaphores instead of the Tile scheduler — avoids Tile's
    start/exit all-engine barrier choreography (~15us fixed cost).

    Engine roles: SP drives the two DMAs; ACT does only Exp (single
    activation-table set -> single table load); GPSIMD computes the
    off-critical-path masks (time>0, relu(ode), W.*W); PE does all matmuls
    and transposes with per-member waits so accumulation groups overlap the
    DVE stream; DVE runs the serial elementwise chain (drain after each op:
    TRN2 DVE has no same-engine RAW interlock).  Cross-engine dependencies
    use per-engine counting semaphores; consumers wait on the producer's
    count (which transitively covers all earlier producers).
    """
    _ensure_repo_on_path()
    from contextlib import ExitStack

    import concourse.bacc as bacc
    import concourse.mybir as mybir

    f32 = mybir.dt.float32
    Alu = mybir.AluOpType
    Act = mybir.ActivationFunctionType
    X = mybir.AxisListType.X

    nc = bacc.Bacc(None, target_bir_lowering=False)
    t_in = nc.dram_tensor("packed", [N, 4 * N + 2], f32,
                          kind="ExternalInput")
    t_out = nc.dram_tensor("flows", [N, N], f32, kind="ExternalOutput")

    with ExitStack() as ctx:
        dma_sem = ctx.enter_context(nc.semaphore("dma_sem"))
        dve_sem = ctx.enter_context(nc.semaphore("dve_sem"))
        pe_sem = ctx.enter_context(nc.semaphore("pe_sem"))
        act_sem = ctx.enter_context(nc.semaphore("act_sem"))
        gp_sem = ctx.enter_context(nc.semaphore("gp_sem"))
        end_sem = ctx.enter_context(nc.semaphore("end_sem"))
        block_cm = nc.Block(no_gpsimd_drain=True)
        block = block_cm.__enter__()

        def sbuf(name, cols=N):
            return ctx.enter_context(nc.sbuf_tensor(name, [N, cols], f32))

        def psum(name, cols=N):
            return ctx.enter_context(nc.psum_tensor(name, [N, cols], f32))

        F32R = os.environ.get("KERNEL_F32R", "0") == "1"

        def _c(ap):
            return ap.bitcast(mybir.dt.float32r) if F32R else ap

        def MM(out, lhsT, rhs, **kw):
            if kw.get("is_transpose"):
                out = _c(out)
            return nc.tensor.matmul(out, _c(lhsT), _c(rhs), **kw)

        def TT(out, in_, ident, **kw):
            return nc.tensor.transpose(_c(out), _c(in_), _c(ident), **kw)

        packed = sbuf("packed_s", 4 * N + 2)
        time_s = packed[:, 0:N]
        ode_s = packed[:, N:2 * N]
        eye_s = packed[:, 2 * N:3 * N]
        offd_s = packed[:, 3 * N:4 * N]
        ones_s = packed[:, 4 * N:4 * N + 1]
        zeros_s = packed[:, 4 * N + 1:4 * N + 2]

        names = ["amask", "wexp", "rode", "W", "WT", "scr", "W2", "W2T",
                 "y2t", "denom0", "wwt", "denom", "maskp", "dsafe", "rec",
                 "dmat", "C0", "C", "CT", "scrh", "ww", "z2t", "q2t", "wwC",
                 "wwtc", "T1", "s1", "s2", "s3", "flows_s"]
        sb = {n: sbuf(n) for n in names}
        nr_v = sbuf("nr", 1)
        h_v = sbuf("h", 1)
        nhg_v = sbuf("nhg", 1)

        p_wt = psum("p_wt")
        p_w2 = psum("p_w2")
        p_w2t = psum("p_w2t")
        p_den = psum("p_den")
        p_ct = psum("p_ct")
        p_t1 = psum("p_t1")
        p_acc = psum("p_acc")
        p_g = psum("p_g", 1)

        # dve_sem values at key producers (hand-counted, asserted below)
        DV = dict(amask=1, W=2, WT=3, scr=4, nr=5, W2=6, W2T=7, y2t=8,
                  denom=11, C0=16, C=17, CT=18, scrh=19, nhg=21, T1=22,
                  wwC=25, flows=30)

        @block.scalar
        def _(scalar):
            scalar.wait_ge(dma_sem, 16)
            nc.scalar.activation(sb["wexp"][:], time_s, Act.Exp,
                                 bias=zeros_s, scale=-lam)\
                .then_inc(act_sem)
            scalar.sem_inc(end_sem, 1)

        @block.gpsimd
        def _(gpsimd):
            gpsimd.wait_ge(dma_sem, 16)
            nc.gpsimd.tensor_scalar_max(sb["rode"][:], ode_s, 0.0)\
                .then_inc(gp_sem)
            gpsimd.wait_ge(dve_sem, DV["W"])
            nc.gpsimd.tensor_mul(sb["ww"][:], sb["W"][:], sb["W"][:])\
                .then_inc(gp_sem)
            gpsimd.sem_inc(end_sem, 1)

        @block.tensor
        def _(tensor):
            tensor.wait_ge(dve_sem, DV["W"])
            TT(p_wt[:], sb["W"][:], eye_s)\
                .then_inc(pe_sem)                                    # pe 1
            tensor.wait_ge(dve_sem, DV["WT"])
            MM(p_w2[:], sb["WT"][:], sb["W"][:], start=True,
                             stop=True).then_inc(pe_sem)             # pe 2
            # W2T = (W^2)^T = (W^T)^2
            MM(p_w2t[:], sb["W"][:], sb["WT"][:], start=True,
                             stop=True).then_inc(pe_sem)             # pe 3
            # p_den = W + W2 + W3 + (W^T .* -r)^T, per-member waits so the
            # group overlaps the DVE stream
            MM(p_den[:], eye_s, sb["W"][:], start=True,
                             stop=False)
            MM(p_den[:], sb["WT"][:], sb["W"][:], start=False,
                             stop=False)
            tensor.wait_ge(dve_sem, DV["W2T"])
            MM(p_den[:], sb["W2T"][:], sb["W"][:], start=False,
                             stop=False)
            tensor.wait_ge(dve_sem, DV["y2t"])
            MM(p_den[:], sb["y2t"][:], eye_s,
                             is_transpose=True, start=False, stop=True)\
                .then_inc(pe_sem, 4)                                 # pe 7
            tensor.wait_ge(dve_sem, DV["C"])
            TT(p_ct[:], sb["C"][:], eye_s)\
                .then_inc(pe_sem)                                    # pe 8
            tensor.wait_ge(dve_sem, DV["scrh"])
            MM(p_g[:], sb["scrh"][:], ones_s, start=True,
                             stop=True).then_inc(pe_sem)             # pe 9
            MM(p_t1[:], sb["CT"][:], sb["WT"][:], start=True,
                             stop=True).then_inc(pe_sem)             # pe 10
            # p_acc = T2 + T3 + T5 + T4 + C + T1
            #         + (-W.*hg)^T + (-C^T.*r)^T + (W.*W.*C)^T
            MM(p_acc[:], sb["W"][:], sb["C"][:], start=True,
                             stop=False)
            MM(p_acc[:], sb["CT"][:], sb["W2T"][:],
                             start=False, stop=False)
            tensor.wait_ge(dve_sem, DV["W2"])
            MM(p_acc[:], sb["W2"][:], sb["C"][:], start=False,
                             stop=False)
            tensor.wait_ge(dve_sem, DV["T1"])
            MM(p_acc[:], sb["W"][:], sb["T1"][:], start=False,
                             stop=False)
            MM(p_acc[:], eye_s, sb["C"][:], start=False,
                             stop=False)
            MM(p_acc[:], eye_s, sb["T1"][:], start=False,
                             stop=False)
            tensor.wait_ge(dve_sem, DV["wwC"])
            MM(p_acc[:], sb["z2t"][:], eye_s,
                             is_transpose=True, start=False, stop=False)
            MM(p_acc[:], sb["q2t"][:], eye_s,
                             is_transpose=True, start=False, stop=False)
            MM(p_acc[:], sb["wwC"][:], eye_s,
                             is_transpose=True, start=False, stop=True)\
                .then_inc(pe_sem, 9)                                 # pe 19
            tensor.sem_inc(end_sem, 1)

        @block.vector
        def _(vector):
            vector.wait_ge(dma_sem, 16)
            nc.vector.tensor_scalar(sb["amask"][:], time_s, 0.0, None,
                                    Alu.is_gt).then_inc(dve_sem)
            nc.vector.drain()                                        # 1
            vector.wait_ge(act_sem, 1)
            nc.vector.tensor_mul(sb["W"][:], sb["wexp"][:], sb["amask"][:])\
                .then_inc(dve_sem)                                   # 2 W
            vector.wait_ge(pe_sem, 1)
            nc.vector.tensor_copy(sb["WT"][:], p_wt[:]).then_inc(dve_sem)
            nc.vector.drain()                                        # 2 WT
            nc.vector.tensor_mul(sb["scr"][:], sb["W"][:], sb["WT"][:])\
                .then_inc(dve_sem)                                   # 3 scr
            nc.vector.drain()
            nc.vector.tensor_reduce(nr_v[:], sb["scr"][:], X, Alu.add,
                                    negate=True).then_inc(dve_sem)   # 4 nr
            vector.wait_ge(pe_sem, 2)
            nc.vector.tensor_copy(sb["W2"][:], p_w2[:]).then_inc(dve_sem)
            vector.wait_ge(pe_sem, 3)
            nc.vector.tensor_copy(sb["W2T"][:], p_w2t[:]).then_inc(dve_sem)
            nc.vector.drain()                                        # 7 W2T
            nc.vector.tensor_scalar(sb["y2t"][:], sb["WT"][:], nr_v[:],
                                    None, Alu.mult).then_inc(dve_sem)
            vector.wait_ge(pe_sem, 7)
            nc.vector.scalar_tensor_tensor(
                sb["denom0"][:], sb["W"][:], nr_v[:], p_den[:], Alu.mult,
                Alu.add)
            nc.vector.tensor_mul(sb["wwt"][:], sb["scr"][:], sb["W"][:])
            nc.vector.drain()                                        # 9
            nc.vector.tensor_add(sb["denom"][:], sb["denom0"][:],
                                 sb["wwt"][:]).then_inc(dve_sem, 3)
            nc.vector.drain()                                        # 10
            nc.vector.tensor_scalar(sb["maskp"][:], sb["denom"][:], 0.0,
                                    None, Alu.is_gt)
            nc.vector.tensor_scalar_add(sb["dsafe"][:], sb["denom"][:],
                                        1e-37)
            nc.vector.drain()                                        # 12
            nc.vector.reciprocal(sb["rec"][:], sb["dsafe"][:])
            vector.wait_ge(gp_sem, 1)
            nc.vector.tensor_mul(sb["dmat"][:], sb["rode"][:], offd_s)
            nc.vector.drain()                                        # 14
            nc.vector.tensor_mul(sb["C0"][:], sb["dmat"][:], sb["rec"][:])\
                .then_inc(dve_sem, 5)
            nc.vector.drain()                                        # 15 C0
            nc.vector.tensor_mul(sb["C"][:], sb["C0"][:], sb["maskp"][:])\
                .then_inc(dve_sem)                                   # 16 C
            vector.wait_ge(pe_sem, 8)
            nc.vector.tensor_copy(sb["CT"][:], p_ct[:]).then_inc(dve_sem)
            nc.vector.drain()                                        # 17 CT
            nc.vector.tensor_mul(sb["scrh"][:], sb["W"][:], sb["C"][:])\
                .then_inc(dve_sem)                                   # 18
            nc.vector.drain()
            nc.vector.tensor_reduce(h_v[:], sb["scrh"][:], X, Alu.add)
            nc.vector.drain()                                        # 19
            vector.wait_ge(pe_sem, 9)
            nc.vector.tensor_scalar(nhg_v[:], h_v[:], p_g[:], -1.0, Alu.add,
                                    Alu.mult).then_inc(dve_sem, 2)
            nc.vector.drain()                                        # 20 nhg
            vector.wait_ge(pe_sem, 10)
            nc.vector.tensor_copy(sb["T1"][:], p_t1[:]).then_inc(dve_sem)
            nc.vector.tensor_scalar(sb["z2t"][:], sb["W"][:], nhg_v[:],
                                    None, Alu.mult)
            nc.vector.tensor_scalar(sb["q2t"][:], sb["CT"][:], nr_v[:],
                                    None, Alu.mult)
            vector.wait_ge(gp_sem, 2)
            nc.vector.tensor_mul(sb["wwC"][:], sb["ww"][:], sb["C"][:])\
                .then_inc(dve_sem, 3)
            nc.vector.tensor_mul(sb["wwtc"][:], sb["scr"][:], sb["C"][:])
            vector.wait_ge(pe_sem, 19)
            nc.vector.scalar_tensor_tensor(
                sb["s1"][:], sb["WT"][:], nhg_v[:], p_acc[:], Alu.mult,
                Alu.add)
            nc.vector.drain()                                        # 26
            nc.vector.scalar_tensor_tensor(
                sb["s2"][:], sb["C"][:], nr_v[:], sb["s1"][:], Alu.mult,
                Alu.add)
            nc.vector.drain()                                        # 27
            nc.vector.scalar_tensor_tensor(
                sb["s3"][:], sb["wwtc"][:], 2.0, sb["s2"][:], Alu.mult,
                Alu.add)
            nc.vector.drain()                                        # 28
            nc.vector.tensor_mul(sb["flows_s"][:], sb["W"][:], sb["s3"][:])\
                .then_inc(dve_sem, 5)
            vector.sem_inc(end_sem, 1)

        @block.sync
        def _(sync):
            sync.dma_start(packed.ap(), t_in[:]).then_inc(dma_sem, 16)
            sync.wait_ge(dve_sem, DV["flows"])
            sync.dma_start(t_out[:], sb["flows_s"][:]).then_inc(dma_sem, 16)
            if os.environ.get("KERNEL_NOWAIT", "0") == "1":
                # Do not hold Sync on out-DMA completion: the walrus epilogue
                # (all-engine barrier + sem-file zeroing) starts ~1.8us sooner
                # and overlaps the in-flight transfer. The epilogue zeroes all
                # sems, so the clears below are redundant in this mode.
                return
            sync.wait_ge(dma_sem, 32)
            # join: by data dependence every other engine retired before the
            # out-DMA completed; clearing the sems here is race-free and
            # makes the NEFF safely re-executable with no all-engine barrier
            sync.wait_ge(end_sem, 4)
            sync.nop()
            if os.environ.get("KERNEL_SIM_NOCLEAR", "0") != "1":
                sync.sem_clear(dma_sem)
                sync.sem_clear(dve_sem)
                sync.sem_clear(pe_sem)
                sync.sem_clear(act_sem)
                sync.sem_clear(gp_sem)
                sync.sem_clear(end_sem)

        block_cm.__exit__(None, None, None)

    # strip the Bass-preamble const-memsets + both all-engine barriers;
    # nothing in this program reads the const tensors, and the counting-sem
    # join above replaces the exit barrier
    drop = {"InstMemset", "InstDrain", "InstEventSemaphore"}
    for blk in nc.m.functions[0].blocks:
        if blk.name == "main" or blk.name.endswith("_end"):
            kept = [i for i in blk.instructions
                    if type(i).__name__ not in drop]
            del blk.instructions[:]
            for i in kept:
                blk.instructions.append(i)

    nc.finalize()
    return nc



def _build_dense_nc_v2(lam):
    """Restructured hand-scheduled dense-flows program (v2).

    Feature flags (env, for HW bisection):
      KERNEL_TTR=1      fused multiply+rowsum via tensor_tensor_reduce
      KERNEL_DBL=1      single [110,220] exp/mask/W ops over time|timeT
      KERNEL_POOLADD=1  eye+W / eye+WT / e2 doubling on Pool (else DVE stt)
      KERNEL_RECIP      "recip" (plain DVE reciprocal, default) | "fast"

    Structure vs v1: WT from exp(-timeT) (no PE transpose round-trip);
    row/col scalings as diag(v) matmul members; BCB = (I+W^T)C(I+W^T)
    collapses four matmuls into two; the only post-p_acc DVE work is two
    ops; no trailing out-DMA wait (the walrus epilogue's sem zeroing
    overlaps the transfer and replaces our cleanup).
    """
    _ensure_repo_on_path()
    from contextlib import ExitStack

    import concourse.bacc as bacc
    import concourse.mybir as mybir

    f32 = mybir.dt.float32
    Alu = mybir.AluOpType
    Act = mybir.ActivationFunctionType
    X = mybir.AxisListType.X

    F_TTR = os.environ.get("KERNEL_TTR", "0") == "1"
    F_DBL = os.environ.get("KERNEL_DBL", "0") == "1"
    F_PADD = os.environ.get("KERNEL_POOLADD", "0") == "1"
    F_FASTR = os.environ.get("KERNEL_RECIP", "recip") == "fast"

    CK = 5 * N + 2  # time | timeT | ode | eye | offd | ones | zeros

    nc = bacc.Bacc(None, target_bir_lowering=False)
    t_in = nc.dram_tensor("packed", [N, CK], f32, kind="ExternalInput")
    t_out = nc.dram_tensor("flows", [N, N], f32, kind="ExternalOutput")

    with ExitStack() as ctx:
        in_sem = ctx.enter_context(nc.semaphore("in_sem"))
        out_sem = ctx.enter_context(nc.semaphore("out_sem"))
        dve_sem = ctx.enter_context(nc.semaphore("dve_sem"))
        pe_sem = ctx.enter_context(nc.semaphore("pe_sem"))
        act_sem = ctx.enter_context(nc.semaphore("act_sem"))
        gp_sem = ctx.enter_context(nc.semaphore("gp_sem"))
        block_cm = nc.Block(no_gpsimd_drain=True)
        block = block_cm.__enter__()

        def sbuf(name, cols=N):
            return ctx.enter_context(nc.sbuf_tensor(name, [N, cols], f32))

        def psum(name, cols=N):
            return ctx.enter_context(nc.psum_tensor(name, [N, cols], f32))

        MM = nc.tensor.matmul
        TT = nc.tensor.transpose

        packed = sbuf("packed_s", CK)
        time_s = packed[:, 0:N]
        timeT_s = packed[:, N:2 * N]
        ode_s = packed[:, 2 * N:3 * N]
        eye_s = packed[:, 3 * N:4 * N]
        offd_s = packed[:, 4 * N:5 * N]
        ones_s = packed[:, 5 * N:5 * N + 1]
        zeros_s = packed[:, 5 * N + 1:5 * N + 2]

        names = ["scr", "diag_nr", "y2tn", "W2s", "W2Ts", "denom", "maskp",
                 "rode", "dmat", "dmm", "rec", "C", "CTs", "scrh",
                 "diag_nhg", "A", "Bs", "wwt", "ww", "wwC", "wwtc", "e2",
                 "t2s", "f0", "s3", "flows_s"]
        sb = {n: sbuf(n) for n in names}
        if F_DBL:
            amask2 = sbuf("amask2", 2 * N)
            wexp2 = sbuf("wexp2", 2 * N)
            Wb = sbuf("Wb", 2 * N)
            W_s = Wb[:, 0:N]
            WT_s = Wb[:, N:2 * N]
        else:
            amask2 = sbuf("amask")
            amaskT = sbuf("amaskT")
            wexp2 = sbuf("wexp")
            wexpT = sbuf("wexpT")
            W_t = sbuf("W")
            WT_t = sbuf("WT")
            W_s = W_t[:]
            WT_s = WT_t[:]
        r_v = sbuf("r_v", 1)
        h_v = sbuf("h_v", 1)
        nhg_v = sbuf("nhg_v", 1)

        p_w2 = psum("p_w2")
        p_w2t = psum("p_w2t")
        p_den = psum("p_den")
        p_ct = psum("p_ct")
        p_g = psum("p_g", 1)
        p_t2 = psum("p_t2")
        p_acc = psum("p_acc")

        # sem counts at key producers (depend on flags; computed in order)
        dv_names = ["amask"]
        if not F_DBL:
            dv_names += ["amaskT"]
        dv_names += ["W"] if F_DBL else ["W", "WT"]
        dv_names += ["scr"]
        if not F_TTR:
            dv_names += ["nr"]
        dv_names += ["diag_nr", "y2tn"]
        if not F_PADD:
            dv_names += ["A", "Bs"]
        dv_names += ["W2s", "W2Ts", "denom", "maskp", "rec", "C", "scrh"]
        if not F_TTR:
            dv_names += ["h"]
        dv_names += ["CTs", "nhg", "diag_nhg", "t2s", "f0", "flows"]
        DV = {n: i + 1 for i, n in enumerate(dv_names)}
        DV["WT"] = DV["W"] if F_DBL else DV["WT"]

        gp_names = ["rode", "dmat"]
        if F_PADD:
            gp_names += ["A", "Bs"]
        gp_names += ["wwt", "ww", "dmm", "wwC", "wwtc"]
        if F_PADD:
            gp_names += ["e2"]
        GP = {n: i + 1 for i, n in enumerate(gp_names)}

        PE = dict(w2=1, w2t=2, den=3, ct=4, g=5, t2=6, acc=7)
        ACTC = dict(wexp=1, wexpT=1 if F_DBL else 2)

        @block.scalar
        def _(scalar):
            scalar.wait_ge(in_sem, 16)
            if F_DBL:
                nc.scalar.activation(wexp2[:], packed[:, 0:2 * N], Act.Exp,
                                     bias=zeros_s, scale=-lam)\
                    .then_inc(act_sem)
            else:
                nc.scalar.activation(wexp2[:], time_s, Act.Exp,
                                     bias=zeros_s, scale=-lam)\
                    .then_inc(act_sem)
                nc.scalar.activation(wexpT[:], timeT_s, Act.Exp,
                                     bias=zeros_s, scale=-lam)\
                    .then_inc(act_sem)

        @block.gpsimd
        def _(gpsimd):
            gpsimd.wait_ge(in_sem, 16)
            nc.gpsimd.tensor_scalar_max(sb["rode"][:], ode_s, 0.0)\
                .then_inc(gp_sem)
            gpsimd.wait_ge(gp_sem, GP["rode"])  # Pool has no RAW interlock
            nc.gpsimd.tensor_mul(sb["dmat"][:], sb["rode"][:], offd_s)\
                .then_inc(gp_sem)
            if F_PADD:
                gpsimd.wait_ge(dve_sem, DV["W"])
                nc.gpsimd.tensor_add(sb["A"][:], eye_s, W_s)\
                    .then_inc(gp_sem)
                gpsimd.wait_ge(dve_sem, DV["WT"])
                nc.gpsimd.tensor_add(sb["Bs"][:], eye_s, WT_s)\
                    .then_inc(gp_sem)
            gpsimd.wait_ge(dve_sem, DV["scr"])
            nc.gpsimd.tensor_mul(sb["wwt"][:], sb["scr"][:], W_s)\
                .then_inc(gp_sem)
            nc.gpsimd.tensor_mul(sb["ww"][:], W_s, W_s).then_inc(gp_sem)
            gpsimd.wait_ge(dve_sem, DV["maskp"])
            gpsimd.wait_ge(gp_sem, GP["dmat"])
            nc.gpsimd.tensor_mul(sb["dmm"][:], sb["dmat"][:],
                                 sb["maskp"][:]).then_inc(gp_sem)
            gpsimd.wait_ge(dve_sem, DV["C"])
            gpsimd.wait_ge(gp_sem, GP["ww"])
            nc.gpsimd.tensor_mul(sb["wwC"][:], sb["ww"][:], sb["C"][:])\
                .then_inc(gp_sem)
            nc.gpsimd.tensor_mul(sb["wwtc"][:], sb["scr"][:], sb["C"][:])\
                .then_inc(gp_sem)
            if F_PADD:
                gpsimd.wait_ge(gp_sem, GP["wwtc"])
                nc.gpsimd.tensor_add(sb["e2"][:], sb["wwtc"][:],
                                     sb["wwtc"][:]).then_inc(gp_sem)

        @block.tensor
        def _(tensor):
            tensor.wait_ge(dve_sem, DV["WT"])
            MM(p_w2[:], WT_s, W_s, start=True, stop=True)\
                .then_inc(pe_sem)                                    # pe 1
            MM(p_w2t[:], W_s, WT_s, start=True, stop=True)\
                .then_inc(pe_sem)                                    # pe 2
            # p_den = W + W2 + diag(-r)W + W3 + (W^T.*-r)^T
            MM(p_den[:], eye_s, W_s, start=True, stop=False)
            MM(p_den[:], WT_s, W_s, start=False, stop=False)
            tensor.wait_ge(dve_sem, DV["diag_nr"])
            MM(p_den[:], sb["diag_nr"][:], W_s, start=False, stop=False)
            tensor.wait_ge(dve_sem, DV["W2s"])
            MM(p_den[:], WT_s, sb["W2s"][:], start=False, stop=False)
            MM(p_den[:], sb["y2tn"][:], eye_s, is_transpose=True,
               start=False, stop=True).then_inc(pe_sem)              # pe 3
            tensor.wait_ge(dve_sem, DV["C"])
            TT(p_ct[:], sb["C"][:], eye_s).then_inc(pe_sem)          # pe 4
            tensor.wait_ge(dve_sem, DV["scrh"])
            MM(p_g[:], sb["scrh"][:], ones_s, start=True, stop=True)\
                .then_inc(pe_sem)                                    # pe 5
            tensor.wait_ge(dve_sem, DV["CTs"])
            tensor.wait_ge(dve_sem if not F_PADD else gp_sem,
                           (DV if not F_PADD else GP)["Bs"])
            MM(p_t2[:], sb["CTs"][:], sb["Bs"][:], start=True, stop=True)\
                .then_inc(pe_sem)                                    # pe 6
            # p_acc = (W2)^T C + diag(-r)C + C diag(-r) + C(W2)^T
            #         + (W.*W.*C)^T + diag(-hg)W^T + W^T diag(-hg) + BCB
            MM(p_acc[:], sb["W2s"][:], sb["C"][:], start=True, stop=False)
            MM(p_acc[:], sb["diag_nr"][:], sb["C"][:], start=False,
               stop=False)
            MM(p_acc[:], sb["CTs"][:], sb["diag_nr"][:], start=False,
               stop=False)
            MM(p_acc[:], sb["CTs"][:], sb["W2Ts"][:], start=False,
               stop=False)
            tensor.wait_ge(gp_sem, GP["wwC"])
            MM(p_acc[:], sb["wwC"][:], eye_s, is_transpose=True,
               start=False, stop=False)
            tensor.wait_ge(dve_sem, DV["diag_nhg"])
            MM(p_acc[:], sb["diag_nhg"][:], WT_s, start=False, stop=False)
            MM(p_acc[:], W_s, sb["diag_nhg"][:], start=False, stop=False)
            tensor.wait_ge(dve_sem, DV["t2s"])
            tensor.wait_ge(dve_sem if not F_PADD else gp_sem,
                           (DV if not F_PADD else GP)["A"])
            MM(p_acc[:], sb["A"][:], sb["t2s"][:], start=False, stop=True)\
                .then_inc(pe_sem)                                    # pe 7

        @block.vector
        def _(vector):
            vector.wait_ge(in_sem, 16)
            if F_DBL:
                nc.vector.tensor_scalar(amask2[:], packed[:, 0:2 * N], 0.0,
                                        None, Alu.is_gt).then_inc(dve_sem)
                nc.vector.drain()
                vector.wait_ge(act_sem, ACTC["wexp"])
                nc.vector.tensor_mul(Wb[:], wexp2[:], amask2[:])\
                    .then_inc(dve_sem)
                nc.vector.drain()
            else:
                nc.vector.tensor_scalar(amask2[:], time_s, 0.0, None,
                                        Alu.is_gt).then_inc(dve_sem)
                nc.vector.tensor_scalar(amaskT[:], timeT_s, 0.0, None,
                                        Alu.is_gt).then_inc(dve_sem)
                nc.vector.drain()
                vector.wait_ge(act_sem, ACTC["wexp"])
                nc.vector.tensor_mul(W_t[:], wexp2[:], amask2[:])\
                    .then_inc(dve_sem)
                vector.wait_ge(act_sem, ACTC["wexpT"])
                nc.vector.tensor_mul(WT_t[:], wexpT[:], amaskT[:])\
                    .then_inc(dve_sem)
                nc.vector.drain()
            if F_TTR:
                nc.vector.tensor_tensor_reduce(
                    sb["scr"][:], W_s, WT_s, 1.0, 0.0, Alu.mult, Alu.add,
                    r_v[:]).then_inc(dve_sem)
                nc.vector.drain()
                nc.vector.tensor_scalar(sb["diag_nr"][:], eye_s, r_v[:],
                                        -1.0, Alu.mult, Alu.mult)\
                    .then_inc(dve_sem)
                nc.vector.tensor_scalar(sb["y2tn"][:], WT_s, r_v[:], -1.0,
                                        Alu.mult, Alu.mult).then_inc(dve_sem)
            else:
                nc.vector.tensor_mul(sb["scr"][:], W_s, WT_s)\
                    .then_inc(dve_sem)
                nc.vector.drain()
                nc.vector.tensor_reduce(r_v[:], sb["scr"][:], X, Alu.add,
                                        negate=True).then_inc(dve_sem)
                nc.vector.drain()
                nc.vector.tensor_scalar(sb["diag_nr"][:], eye_s, r_v[:],
                                        None, Alu.mult).then_inc(dve_sem)
                nc.vector.tensor_scalar(sb["y2tn"][:], WT_s, r_v[:], None,
                                        Alu.mult).then_inc(dve_sem)
            if not F_PADD:
                nc.vector.tensor_add(sb["A"][:], eye_s, W_s)\
                    .then_inc(dve_sem)
                nc.vector.tensor_add(sb["Bs"][:], eye_s, WT_s)\
                    .then_inc(dve_sem)
            vector.wait_ge(pe_sem, PE["w2"])
            nc.vector.tensor_copy(sb["W2s"][:], p_w2[:]).then_inc(dve_sem)
            vector.wait_ge(pe_sem, PE["w2t"])
            nc.vector.tensor_copy(sb["W2Ts"][:], p_w2t[:]).then_inc(dve_sem)
            vector.wait_ge(pe_sem, PE["den"])
            vector.wait_ge(gp_sem, GP["wwt"])
            # denom + 1e-37 fused: no-path entries end exactly at 1e-37
            nc.vector.scalar_tensor_tensor(
                sb["denom"][:], sb["wwt"][:], 1e-37, p_den[:], Alu.add,
                Alu.add).then_inc(dve_sem)
            nc.vector.drain()
            nc.vector.tensor_scalar(sb["maskp"][:], sb["denom"][:], 1e-20,
                                    None, Alu.is_gt).then_inc(dve_sem)
            if F_FASTR:
                nc.vector.reciprocal_approx_fast(sb["rec"][:],
                                                 sb["denom"][:])\
                    .then_inc(dve_sem)
            else:
                nc.vector.reciprocal(sb["rec"][:], sb["denom"][:])\
                    .then_inc(dve_sem)
            nc.vector.drain()
            vector.wait_ge(gp_sem, GP["dmm"])
            nc.vector.tensor_mul(sb["C"][:], sb["dmm"][:], sb["rec"][:])\
                .then_inc(dve_sem)
            nc.vector.drain()
            if F_TTR:
                nc.vector.tensor_tensor_reduce(
                    sb["scrh"][:], W_s, sb["C"][:], 1.0, 0.0, Alu.mult,
                    Alu.add, h_v[:]).then_inc(dve_sem)
            else:
                nc.vector.tensor_mul(sb["scrh"][:], W_s, sb["C"][:])\
                    .then_inc(dve_sem)
                nc.vector.drain()
                nc.vector.tensor_reduce(h_v[:], sb["scrh"][:], X, Alu.add)\
                    .then_inc(dve_sem)
            vector.wait_ge(pe_sem, PE["ct"])
            nc.vector.tensor_copy(sb["CTs"][:], p_ct[:]).then_inc(dve_sem)
            nc.vector.drain()
            vector.wait_ge(pe_sem, PE["g"])
            nc.vector.tensor_scalar(nhg_v[:], h_v[:], p_g[:], -1.0, Alu.add,
                                    Alu.mult).then_inc(dve_sem)
            nc.vector.drain()
            nc.vector.tensor_scalar(sb["diag_nhg"][:], eye_s, nhg_v[:],
                                    None, Alu.mult).then_inc(dve_sem)
            vector.wait_ge(pe_sem, PE["t2"])
            nc.vector.tensor_copy(sb["t2s"][:], p_t2[:]).then_inc(dve_sem)
            vector.wait_ge(pe_sem, PE["acc"])
            if F_PADD:
                nc.vector.tensor_mul(sb["f0"][:], W_s, p_acc[:])\
                    .then_inc(dve_sem)
                nc.vector.drain()
                vector.wait_ge(gp_sem, GP["e2"])
                nc.vector.tensor_mul(sb["s3"][:], W_s, sb["e2"][:])
                nc.vector.drain()
                nc.vector.tensor_add(sb["flows_s"][:], sb["f0"][:],
                                     sb["s3"][:]).then_inc(dve_sem)
            else:
                vector.wait_ge(gp_sem, GP["wwtc"])
                nc.vector.scalar_tensor_tensor(
                    sb["s3"][:], sb["wwtc"][:], 2.0, p_acc[:], Alu.mult,
                    Alu.add).then_inc(dve_sem)
                nc.vector.drain()
                nc.vector.tensor_mul(sb["flows_s"][:], W_s, sb["s3"][:])\
                    .then_inc(dve_sem)

        @block.sync
        def _(sync):
            sync.dma_start(packed.ap(), t_in[:]).then_inc(in_sem, 16)
            sync.wait_ge(dve_sem, DV["flows"])
            # out-DMA completion sem is required by codegen but unwaited:
            # the walrus epilogue overlaps the in-flight transfer and its
            # sem-file zeroing replaces any cleanup of ours
            sync.dma_start(t_out[:], sb["flows_s"][:]).then_inc(out_sem, 16)
            if os.environ.get("KERNEL_V2WAIT", "0") == "1":
                sync.wait_ge(out_sem, 16)

        block_cm.__exit__(None, None, None)

    drop = {"InstMemset", "InstDrain", "InstEventSemaphore"}
    for blk in nc.m.functions[0].blocks:
        if blk.name == "main" or blk.name.endswith("_end"):
            kept = [i for i in blk.instructions
                    if type(i).__name__ not in drop]
            del blk.instructions[:]
            for i in kept:
                blk.instructions.append(i)

    nc.finalize()
    return nc


def _build_dense_nc(lam):
    """Trace the dense-flows Bass program (compile-time constant lambda).

    Engine split: ACT does the LUT ops (sign/exp/relu/square), PE does all
    matmuls/transposes with maximal PSUM accumulation (incl. accumulating
    transposes), DVE does the remaining elementwise stream (~28 ops).
    """
    _ensure_repo_on_path()
    import concourse.bacc as bacc
    import concourse.mybir as mybir
    import concourse.tile as tile

    f32 = mybir.dt.float32
    Alu = mybir.AluOpType
    Act = mybir.ActivationFunctionType
    X = mybir.AxisListType.X

    nc = bacc.Bacc(None, target_bir_lowering=False)
    # packed input: [:, 0:N]=time_mat, [:, N:2N]=ode, [:, 2N:3N]=eye,
    # [:, 3N:4N]=offdiag, [:, 4N]=ones — one DMA (one sync wait) loads all
    t_in = nc.dram_tensor("packed", [N, 4 * N + 1], f32, kind="ExternalInput")
    t_out = nc.dram_tensor("flows", [N, N], f32, kind="ExternalOutput")

    with tile.TileContext(nc) as tc:
        with (
            tc.tile_pool(name="sbuf", bufs=1) as pool,
            tc.tile_pool(name="psum", bufs=1, space="PSUM") as psum,
        ):
            def sb(tag):
                return pool.tile([N, N], f32, name=tag, tag=tag)

            def ps(tag, shared=True):
                # 8 PSUM banks: transient matmul/transpose outputs rotate
                # through 3 shared slots; accumulation groups get their own
                t = "pp" if shared else tag
                return psum.tile([N, N], f32, name=tag, tag=t,
                                 bufs=(3 if shared else 1))

            packed_s = pool.tile([N, 4 * N + 1], f32, name="packed",
                                 tag="packed")
            nc.sync.dma_start(packed_s[:], t_in[:])
            time_s = packed_s[:, 0:N]
            ode_s = packed_s[:, N:2 * N]
            eye_s = packed_s[:, 2 * N:3 * N]
            offd_s = packed_s[:, 3 * N:4 * N]
            ones_s = packed_s[:, 4 * N:4 * N + 1]

            # W = exp(-lam*time) .* (time > 0)
            amask = sb("amask")
            nc.scalar.activation(amask[:], time_s[:], Act.Sign)
            wexp = sb("wexp")
            nc.scalar.activation(wexp[:], time_s[:], Act.Exp, scale=-lam)
            W = sb("W")
            nc.vector.tensor_mul(W[:], wexp[:], amask[:])

            p_wt = ps("p_wt")
            nc.tensor.transpose(p_wt[:], W[:], eye_s[:])
            WT = sb("WT")
            nc.vector.tensor_copy(WT[:], p_wt[:])

            scr = sb("scr")                    # W .* W^T (reused 3x)
            nc.vector.tensor_mul(scr[:], W[:], WT[:])
            nr_v = pool.tile([N, 1], f32, name="nr", tag="nr")
            nc.vector.tensor_reduce(nr_v[:], scr[:], X, Alu.add, negate=True)

            p_w2 = ps("p_w2")
            nc.tensor.matmul(p_w2[:], WT[:], W[:], start=True, stop=True)
            W2 = sb("W2")
            nc.vector.tensor_copy(W2[:], p_w2[:])
            p_w2t = ps("p_w2t")
            nc.tensor.transpose(p_w2t[:], W2[:], eye_s[:])
            W2T = sb("W2T")
            nc.vector.tensor_copy(W2T[:], p_w2t[:])

            # denom = W + W2 + W3 - W.*(r(+)r) + W.*W.*W^T, accumulated on PE:
            # p_den = W + W2 + W3 + (W^T.*(-r))^T  (last term = -W.*r_cols)
            y2t = sb("y2t")
            nc.vector.tensor_scalar(y2t[:], WT[:], nr_v[:], None, Alu.mult)
            p_den = ps("p_den", shared=False)
            nc.tensor.matmul(p_den[:], eye_s[:], W[:], start=True, stop=False)
            nc.tensor.matmul(p_den[:], eye_s[:], W2[:], start=False,
                             stop=False)
            nc.tensor.matmul(p_den[:], W2T[:], W[:], start=False, stop=False)
            nc.tensor.matmul(p_den[:], y2t[:], eye_s[:], is_transpose=True,
                             start=False, stop=True)
            denom0 = sb("denom0")
            nc.vector.scalar_tensor_tensor(
                denom0[:], W[:], nr_v[:], p_den[:], Alu.mult, Alu.add)
            wwt = sb("wwt")                    # scr .* W = W.*W.*W^T
            nc.vector.tensor_mul(wwt[:], scr[:], W[:])
            denom = sb("denom")
            nc.vector.tensor_add(denom[:], denom0[:], wwt[:])

            # C = D / denom (0 where denom == 0)
            maskp = sb("maskp")
            nc.scalar.activation(maskp[:], denom[:], Act.Sign)
            dsafe = sb("dsafe")
            nc.vector.tensor_scalar_add(dsafe[:], denom[:], 1e-37)
            rec = sb("rec")
            nc.vector.reciprocal(rec[:], dsafe[:])
            rode = sb("rode")
            nc.scalar.activation(rode[:], ode_s[:], Act.Relu)
            dmat = sb("dmat")
            nc.vector.tensor_mul(dmat[:], rode[:], offd_s[:])
            C0 = sb("C0")
            nc.vector.tensor_mul(C0[:], dmat[:], rec[:])
            C = sb("C")
            nc.vector.tensor_mul(C[:], C0[:], maskp[:])

            p_ct = ps("p_ct")
            nc.tensor.transpose(p_ct[:], C[:], eye_s[:])
            CT = sb("CT")
            nc.vector.tensor_copy(CT[:], p_ct[:])

            # h = rowsum(W.*C); g = colsum(W.*C) via PE; nhg = -(h+g)
            scrh = sb("scrh")
            nc.vector.tensor_mul(scrh[:], W[:], C[:])
            h_v = pool.tile([N, 1], f32, name="h", tag="h")
            nc.vector.tensor_reduce(h_v[:], scrh[:], X, Alu.add)
            p_g = psum.tile([N, 1], f32, name="p_g", tag="p_g", bufs=1)
            nc.tensor.matmul(p_g[:], scrh[:], ones_s[:], start=True,
                             stop=True)
            nhg_v = pool.tile([N, 1], f32, name="nhg", tag="nhg")
            nc.vector.tensor_scalar(nhg_v[:], h_v[:], p_g[:], -1.0, Alu.add,
                                    Alu.mult)

            # T1 = C @ W^T (standalone: rhs of T4)
            p_t1 = ps("p_t1")
            nc.tensor.matmul(p_t1[:], CT[:], WT[:], start=True, stop=True)
            T1 = sb("T1")
            nc.vector.tensor_copy(T1[:], p_t1[:])

            ww = sb("ww")
            nc.scalar.activation(ww[:], W[:], Act.Square)
            z2t = sb("z2t")
            nc.vector.tensor_scalar(z2t[:], W[:], nhg_v[:], None, Alu.mult)
            q2t = sb("q2t")
            nc.vector.tensor_scalar(q2t[:], CT[:], nr_v[:], None, Alu.mult)
            wwC = sb("wwC")
            nc.vector.tensor_mul(wwC[:], ww[:], C[:])

            # p_acc = T2 + T3 + T5 + T4 + C + T1
            #         + (-W.*hg)^T + (-C^T.*r)^T + (W.*W.*C)^T
            p_acc = ps("p_acc", shared=False)
            nc.tensor.matmul(p_acc[:], W[:], C[:], start=True, stop=False)
            nc.tensor.matmul(p_acc[:], CT[:], W2T[:], start=False, stop=False)
            nc.tensor.matmul(p_acc[:], W2[:], C[:], start=False, stop=False)
            nc.tensor.matmul(p_acc[:], W[:], T1[:], start=False, stop=False)
            nc.tensor.matmul(p_acc[:], eye_s[:], C[:], start=False,
                             stop=False)
            nc.tensor.matmul(p_acc[:], eye_s[:], T1[:], start=False,
                             stop=False)
            nc.tensor.matmul(p_acc[:], z2t[:], eye_s[:], is_transpose=True,
                             start=False, stop=False)
            nc.tensor.matmul(p_acc[:], q2t[:], eye_s[:], is_transpose=True,
                             start=False, stop=False)
            nc.tensor.matmul(p_acc[:], wwC[:], eye_s[:], is_transpose=True,
                             start=False, stop=True)

            s1 = sb("s1")
            nc.vector.scalar_tensor_tensor(
                s1[:], WT[:], nhg_v[:], p_acc[:], Alu.mult, Alu.add)
            s2 = sb("s2")
            nc.vector.scalar_tensor_tensor(
                s2[:], C[:], nr_v[:], s1[:], Alu.mult, Alu.add)
            wwtc = sb("wwtc")
            nc.vector.tensor_mul(wwtc[:], scr[:], C[:])
            s3 = sb("s3")
            nc.vector.scalar_tensor_tensor(
                s3[:], wwtc[:], 2.0, s2[:], Alu.mult, Alu.add)
            flows_s = sb("flows")
            nc.vector.tensor_mul(flows_s[:], W[:], s3[:])
            nc.sync.dma_start(t_out[:], flows_s[:])

    nc.finalize()
    return nc


def _install_ntff_shim():
    """Best-effort NTFF profiling hook for trace mode (KERNEL_TRACE=1).

    The agent image's antenv lacks axon_hooks; recreate it and register the
    ctypes profiler from the axon boot script so run_bass_kernel_spmd's
    trace path (neuron-profile on the NTFFs) works.
    """
    import types
    try:
        import antenv
        if "antenv.axon_hooks" not in sys.modules:
            mod = types.ModuleType("antenv.axon_hooks")
            _h = [None]
            mod.set_axon_ntff_profile_hook = lambda h: _h.__setitem__(0, h)
            mod.get_axon_ntff_profile_hook = lambda: _h[0]
            sys.modules["antenv.axon_hooks"] = mod
            antenv.axon_hooks = mod
        if sys.modules["antenv.axon_hooks"].get_axon_ntff_profile_hook() \
                is None:
            if "/root/.axon_site" not in sys.path:
                sys.path.insert(0, "/root/.axon_site")
            from trn_agent_boot.trn_boot import _ntff_profile_via_ctypes
            sys.modules["antenv.axon_hooks"].set_axon_ntff_profile_hook(
                _ntff_profile_via_ctypes("/opt/axon/libaxon_pjrt.so"))
        from concourse import bass_utils
        bass_utils.upload_artifacts = lambda tmpdir: f"file://{tmpdir}"
        return True
    except Exception:
        return False


def _run_dense(ode, time_mat, lam):
    global LAST_EXEC_NS
    _ensure_repo_on_path()
    from concourse.bass_utils import run_bass_kernel_spmd

    impl = os.environ.get("KERNEL_IMPL", "v2")
    key = (float(lam), impl)
    if key not in _NC_CACHE:
        builder = {"v2": _build_dense_nc_v2, "raw": _build_dense_nc_raw,
                   "tile": _build_dense_nc}[impl]
        _NC_CACHE[key] = builder(float(lam))
    nc = _NC_CACHE[key]

    if impl == "v2":
        packed = np.zeros((N, 5 * N + 2), np.float32)
        packed[:, 0:N] = time_mat
        packed[:, N:2 * N] = time_mat.T
        packed[:, 2 * N:3 * N] = ode
        packed[:, 3 * N:4 * N] = np.eye(N, dtype=np.float32)
        packed[:, 4 * N:5 * N] = 1.0 - np.eye(N, dtype=np.float32)
        packed[:, 5 * N] = 1.0
    else:
        packed = np.zeros((N, 4 * N + (2 if impl == "raw" else 1)),
                          np.float32)
        packed[:, 0:N] = time_mat
        packed[:, N:2 * N] = ode
        packed[:, 2 * N:3 * N] = np.eye(N, dtype=np.float32)
        packed[:, 3 * N:4 * N] = 1.0 - np.eye(N, dtype=np.float32)
        packed[:, 4 * N] = 1.0
    in_map = {"packed": packed}
    n_cores = 8
    trace = os.environ.get("KERNEL_TRACE", "0") == "1"
    kwargs = {}
    if trace:
        trace = _install_ntff_shim()
        if trace:
            import tempfile
            kwargs["tmpdir"] = tempfile.mkdtemp(prefix="bass_ntff_")
    res = run_bass_kernel_spmd(
        nc, [dict(in_map) for _ in range(n_cores)], list(range(n_cores)),
        trace=trace, **kwargs)
    LAST_EXEC_NS = res.exec_time_ns
    return np.asarray(res.results[0]["flows"], np.float32)


# --------------------------------------------------------------------------
# Fallback: faithful elementwise computation (non-conforming inputs only)
# --------------------------------------------------------------------------

def _fallback(ode, time_mat, path_u, path_v, edge_mask, od_o, od_d, group,
              lam):
    n = ode.shape[0]
    nseg = n * n + 1
    m = edge_mask.astype(np.float64)
    t_p = (time_mat.astype(np.float64)[path_u, path_v] * m).sum(-1)
    logits = -lam * t_p
    gmax = np.full(nseg, -np.inf)
    np.maximum.at(gmax, group, logits)
    gmax = np.where(np.isfinite(gmax), gmax, 0.0)
    e = np.exp(logits - gmax[group])
    den = np.zeros(nseg)
    np.add.at(den, group, e)
    den_safe = np.where(den > 0, den, 1.0)
    probs = e / den_safe[group]
    demand = ode.astype(np.float64)[od_o, od_d]
    demand = np.where((demand > 0) & (od_o != od_d), demand, 0.0)
    contrib = (demand * probs)[:, None] * m
    flat = (path_u.astype(np.int64) * n + path_v).reshape(-1)
    flows = np.zeros(n * n)
    np.add.at(flows, flat, contrib.reshape(-1))
    return flows.reshape(n, n).astype(time_mat.dtype)


# --------------------------------------------------------------------------


def kernel(ode, time_mat, path_u, path_v, edge_mask, od_o, od_d, group,
           lambda_param):
    ode = np.asarray(ode)
    time_mat = np.asarray(time_mat)
    path_u = np.asarray(path_u)
    path_v = np.asarray(path_v)
    edge_mask = np.asarray(edge_mask)
    od_o = np.asarray(od_o)
    od_d = np.asarray(od_d)
    group = np.asarray(group)
    lam = float(np.asarray(lambda_param))

    if 0.0 <= lam <= 8.0 and _inputs_conform(
            time_mat, path_u, path_v, edge_mask, od_o, od_d, group):
        return _run_dense(ode, time_mat, lam)
    return _fallback(ode, time_mat, path_u, path_v, edge_mask, od_o, od_d,
                     group, lam)



# revision 17
# speedup vs baseline: 1.0389x; 1.0389x over previous
"""Trainium2 kernel for nn_BilevelFramework (path-based traffic assignment).

The oracle's inputs enumerate ALL simple paths of <=3 edges of the directed
graph encoded by ``time_mat`` (edge exists iff time > 0), grouped per OD pair
(group = o*N + d), padded to P_MAX with a dummy segment. For such inputs the
per-OD softmax over paths and the edge scatter-add collapse exactly into
dense 110x110 matrix algebra over W = exp(-lambda*time) .* (time>0):

  denom       = W + W^2 + W^3 - W.*(r(+)r) + W.*W.*W^T          (r = diag(W^2))
  C           = D / denom,  D = ode .* (ode>0) .* offdiag
  flows = W .* ( C + C W^T + W^T C + C (W^2)^T + W^T C W^T + (W^2)^T C
                 - W^T.*((h+g)(+)(h+g)) - C.*(r(+)r)
                 + 2 W.*W^T.*C + (W^T.*W^T).*C^T )
  with h = rowsum(W.*C), g = colsum(W.*C).

(The inclusion-exclusion terms remove non-simple paths, exactly matching the
reference's path enumeration constraints; verified to ~1e-15 rel in float64
and ~1e-6 in float32 against the oracle.)

The kernel verifies on the host that the path inputs are exactly that
enumeration (order-independent multiset check). If they are, it runs the
dense computation on the TRN2 NeuronCores via a Bass/Tile kernel (SPMD on
cores 0-7). Otherwise it falls back to a faithful elementwise computation.
"""

import os
import sys

import numpy as np

N = 110
P_MAX = 400000
NSEG = N * N + 1

LAST_EXEC_NS = None  # filled when KERNEL_TRACE=1


# --------------------------------------------------------------------------
# Host-side structure check: inputs == full <=3-edge simple-path enumeration?
# --------------------------------------------------------------------------

def _enumerate_records(A):
    """Record table [P, 12] (u0..2, v0..2, m0..2, oo, dd, g) of the full
    <=3-edge simple-path enumeration of adjacency A, or None if it would
    overflow P_MAX (the reference would truncate, which we don't model)."""
    idx = np.arange(N, dtype=np.int32)

    o1, d1 = np.nonzero(A)
    o1 = o1.astype(np.int32)
    d1 = d1.astype(np.int32)

    B2 = A[:, :, None] & A[None, :, :]
    B2 &= idx[:, None, None] != idx[None, None, :]  # o != d
    o2, k2, d2 = [x.astype(np.int32) for x in np.nonzero(B2)]

    B3 = (A[:, :, None, None] & A[None, :, :, None]) & A[None, None, :, :]
    B3 &= idx[:, None, None, None] != idx[None, None, :, None]  # o != b
    B3 &= idx[None, :, None, None] != idx[None, None, None, :]  # a != d
    B3 &= idx[:, None, None, None] != idx[None, None, None, :]  # o != d
    o3, a3, b3, d3 = [x.astype(np.int32) for x in np.nonzero(B3)]

    n1, n2, n3 = len(o1), len(o2), len(o3)
    total = n1 + n2 + n3
    if total > P_MAX:
        return None

    rec = np.zeros((P_MAX, 12), np.int32)
    rec[:, 11] = N * N  # padding group
    ofs = 0
    # 1-edge
    rec[ofs:ofs + n1, 0] = o1
    rec[ofs:ofs + n1, 3] = d1
    rec[ofs:ofs + n1, 6] = 1
    rec[ofs:ofs + n1, 9] = o1
    rec[ofs:ofs + n1, 10] = d1
    rec[ofs:ofs + n1, 11] = o1 * N + d1
    ofs += n1
    # 2-edge
    rec[ofs:ofs + n2, 0] = o2
    rec[ofs:ofs + n2, 1] = k2
    rec[ofs:ofs + n2, 3] = k2
    rec[ofs:ofs + n2, 4] = d2
    rec[ofs:ofs + n2, 6] = 1
    rec[ofs:ofs + n2, 7] = 1
    rec[ofs:ofs + n2, 9] = o2
    rec[ofs:ofs + n2, 10] = d2
    rec[ofs:ofs + n2, 11] = o2 * N + d2
    ofs += n2
    # 3-edge
    rec[ofs:ofs + n3, 0] = o3
    rec[ofs:ofs + n3, 1] = a3
    rec[ofs:ofs + n3, 2] = b3
    rec[ofs:ofs + n3, 3] = a3
    rec[ofs:ofs + n3, 4] = b3
    rec[ofs:ofs + n3, 5] = d3
    rec[ofs:ofs + n3, 6:9] = 1
    rec[ofs:ofs + n3, 9] = o3
    rec[ofs:ofs + n3, 10] = d3
    rec[ofs:ofs + n3, 11] = o3 * N + d3
    return rec


def _sort_rows(rec):
    # lexsort by all 12 columns (column 0 = most significant; any fixed
    # total order works for multiset comparison)
    order = np.lexsort(tuple(rec[:, c] for c in range(11, -1, -1)))
    return rec[order]


def _inputs_conform(time_mat, path_u, path_v, edge_mask, od_o, od_d, group):
    if (path_u.shape != (P_MAX, 3) or path_v.shape != (P_MAX, 3)
            or edge_mask.shape != (P_MAX, 3) or od_o.shape != (P_MAX,)
            or od_d.shape != (P_MAX,) or group.shape != (P_MAX,)
            or time_mat.shape != (N, N)):
        return False
    if np.any(np.diag(time_mat) != 0.0):
        return False
    if np.any(time_mat < 0.0):
        return False
    A = time_mat > 0.0
    rec = _enumerate_records(A)
    if rec is None:
        return False
    given = np.zeros((P_MAX, 12), np.int32)
    given[:, 0:3] = path_u
    given[:, 3:6] = path_v
    given[:, 6:9] = edge_mask
    given[:, 9] = od_o
    given[:, 10] = od_d
    given[:, 11] = group
    return bool(np.array_equal(_sort_rows(rec), _sort_rows(given)))


# --------------------------------------------------------------------------
# Dense Bass/Tile device kernel
# --------------------------------------------------------------------------

def _ensure_repo_on_path():
    try:
        import concourse  # noqa: F401
    except ImportError:
        for p in ("/opt/trn_rl_repo", os.path.expanduser("~/trn_rl_repo")):
            if os.path.isdir(p):
                sys.path.insert(0, p)
                break


_NC_CACHE = {}


def _build_dense_nc_raw(lam):
    """Hand-scheduled (raw bacc) dense-flows program.

    Same math as _build_dense_nc, but explicit per-engine streams and
    counting semaphores instead of the Tile scheduler — avoids Tile's
    start/exit all-engine barrier choreography (~15us fixed cost).

    Engine roles: SP drives the two DMAs; ACT does only Exp (single
    activation-table set -> single table load); GPSIMD computes the
    off-critical-path masks (time>0, relu(ode), W.*W); PE does all matmuls
    and transposes with per-member waits so accumulation groups overlap the
    DVE stream; DVE runs the serial elementwise chain (drain after each op:
    TRN2 DVE has no same-engine RAW interlock).  Cross-engine dependencies
    use per-engine counting semaphores; consumers wait on the producer's
    count (which transitively covers all earlier producers).
    """
    _ensure_repo_on_path()
    from contextlib import ExitStack

    import concourse.bacc as bacc
    import concourse.mybir as mybir

    f32 = mybir.dt.float32
    Alu = mybir.AluOpType
    Act = mybir.ActivationFunctionType
    X = mybir.AxisListType.X

    nc = bacc.Bacc(None, target_bir_lowering=False)
    t_in = nc.dram_tensor("packed", [N, 4 * N + 2], f32,
                          kind="ExternalInput")
    t_out = nc.dram_tensor("flows", [N, N], f32, kind="ExternalOutput")

    with ExitStack() as ctx:
        dma_sem = ctx.enter_context(nc.semaphore("dma_sem"))
        dve_sem = ctx.enter_context(nc.semaphore("dve_sem"))
        pe_sem = ctx.enter_context(nc.semaphore("pe_sem"))
        act_sem = ctx.enter_context(nc.semaphore("act_sem"))
        gp_sem = ctx.enter_context(nc.semaphore("gp_sem"))
        end_sem = ctx.enter_context(nc.semaphore("end_sem"))
        block_cm = nc.Block(no_gpsimd_drain=True)
        block = block_cm.__enter__()

        def sbuf(name, cols=N):
            return ctx.enter_context(nc.sbuf_tensor(name, [N, cols], f32))

        def psum(name, cols=N):
            return ctx.enter_context(nc.psum_tensor(name, [N, cols], f32))

        F32R = os.environ.get("KERNEL_F32R", "0") == "1"

        def _c(ap):
            return ap.bitcast(mybir.dt.float32r) if F32R else ap

        def MM(out, lhsT, rhs, **kw):
            if kw.get("is_transpose"):
                out = _c(out)
            return nc.tensor.matmul(out, _c(lhsT), _c(rhs), **kw)

        def TT(out, in_, ident, **kw):
            return nc.tensor.transpose(_c(out), _c(in_), _c(ident), **kw)

        packed = sbuf("packed_s", 4 * N + 2)
        time_s = packed[:, 0:N]
        ode_s = packed[:, N:2 * N]
        eye_s = packed[:, 2 * N:3 * N]
        offd_s = packed[:, 3 * N:4 * N]
        ones_s = packed[:, 4 * N:4 * N + 1]
        zeros_s = packed[:, 4 * N + 1:4 * N + 2]

        names = ["amask", "wexp", "rode", "W", "WT", "scr", "W2", "W2T",
                 "y2t", "denom0", "wwt", "denom", "maskp", "dsafe", "rec",
                 "dmat", "C0", "C", "CT", "scrh", "ww", "z2t", "q2t", "wwC",
                 "wwtc", "T1", "s1", "s2", "s3", "flows_s"]
        sb = {n: sbuf(n) for n in names}
        nr_v = sbuf("nr", 1)
        h_v = sbuf("h", 1)
        nhg_v = sbuf("nhg", 1)

        p_wt = psum("p_wt")
        p_w2 = psum("p_w2")
        p_w2t = psum("p_w2t")
        p_den = psum("p_den")
        p_ct = psum("p_ct")
        p_t1 = psum("p_t1")
        p_acc = psum("p_acc")
        p_g = psum("p_g", 1)

        # dve_sem values at key producers (hand-counted, asserted below)
        DV = dict(amask=1, W=2, WT=3, scr=4, nr=5, W2=6, W2T=7, y2t=8,
                  denom=11, C0=16, C=17, CT=18, scrh=19, nhg=21, T1=22,
                  wwC=25, flows=30)

        @block.scalar
        def _(scalar):
            scalar.wait_ge(dma_sem, 16)
            nc.scalar.activation(sb["wexp"][:], time_s, Act.Exp,
                                 bias=zeros_s, scale=-lam)\
                .then_inc(act_sem)
            scalar.sem_inc(end_sem, 1)

        @block.gpsimd
        def _(gpsimd):
            gpsimd.wait_ge(dma_sem, 16)
            nc.gpsimd.tensor_scalar_max(sb["rode"][:], ode_s, 0.0)\
                .then_inc(gp_sem)
            gpsimd.wait_ge(dve_sem, DV["W"])
            nc.gpsimd.tensor_mul(sb["ww"][:], sb["W"][:], sb["W"][:])\
                .then_inc(gp_sem)
            gpsimd.sem_inc(end_sem, 1)

        @block.tensor
        def _(tensor):
            tensor.wait_ge(dve_sem, DV["W"])
            TT(p_wt[:], sb["W"][:], eye_s)\
                .then_inc(pe_sem)                                    # pe 1
            tensor.wait_ge(dve_sem, DV["WT"])
            MM(p_w2[:], sb["WT"][:], sb["W"][:], start=True,
                             stop=True).then_inc(pe_sem)             # pe 2
            # W2T = (W^2)^T = (W^T)^2
            MM(p_w2t[:], sb["W"][:], sb["WT"][:], start=True,
                             stop=True).then_inc(pe_sem)             # pe 3
            # p_den = W + W2 + W3 + (W^T .* -r)^T, per-member waits so the
            # group overlaps the DVE stream
            MM(p_den[:], eye_s, sb["W"][:], start=True,
                             stop=False)
            MM(p_den[:], sb["WT"][:], sb["W"][:], start=False,
                             stop=False)
            tensor.wait_ge(dve_sem, DV["W2T"])
            MM(p_den[:], sb["W2T"][:], sb["W"][:], start=False,
                             stop=False)
            tensor.wait_ge(dve_sem, DV["y2t"])
            MM(p_den[:], sb["y2t"][:], eye_s,
                             is_transpose=True, start=False, stop=True)\
                .then_inc(pe_sem, 4)                                 # pe 7
            tensor.wait_ge(dve_sem, DV["C"])
            TT(p_ct[:], sb["C"][:], eye_s)\
                .then_inc(pe_sem)                                    # pe 8
            tensor.wait_ge(dve_sem, DV["scrh"])
            MM(p_g[:], sb["scrh"][:], ones_s, start=True,
                             stop=True).then_inc(pe_sem)             # pe 9
            MM(p_t1[:], sb["CT"][:], sb["WT"][:], start=True,
                             stop=True).then_inc(pe_sem)             # pe 10
            # p_acc = T2 + T3 + T5 + T4 + C + T1
            #         + (-W.*hg)^T + (-C^T.*r)^T + (W.*W.*C)^T
            MM(p_acc[:], sb["W"][:], sb["C"][:], start=True,
                             stop=False)
            MM(p_acc[:], sb["CT"][:], sb["W2T"][:],
                             start=False, stop=False)
            tensor.wait_ge(dve_sem, DV["W2"])
            MM(p_acc[:], sb["W2"][:], sb["C"][:], start=False,
                             stop=False)
            tensor.wait_ge(dve_sem, DV["T1"])
            MM(p_acc[:], sb["W"][:], sb["T1"][:], start=False,
                             stop=False)
            MM(p_acc[:], eye_s, sb["C"][:], start=False,
                             stop=False)
            MM(p_acc[:], eye_s, sb["T1"][:], start=False,
                             stop=False)
            tensor.wait_ge(dve_sem, DV["wwC"])
            MM(p_acc[:], sb["z2t"][:], eye_s,
                             is_transpose=True, start=False, stop=False)
            MM(p_acc[:], sb["q2t"][:], eye_s,
                             is_transpose=True, start=False, stop=False)
            MM(p_acc[:], sb["wwC"][:], eye_s,
                             is_transpose=True, start=False, stop=True)\
                .then_inc(pe_sem, 9)                                 # pe 19
            tensor.sem_inc(end_sem, 1)

        @block.vector
        def _(vector):
            vector.wait_ge(dma_sem, 16)
            nc.vector.tensor_scalar(sb["amask"][:], time_s, 0.0, None,
                                    Alu.is_gt).then_inc(dve_sem)
            nc.vector.drain()                                        # 1
            vector.wait_ge(act_sem, 1)
            nc.vector.tensor_mul(sb["W"][:], sb["wexp"][:], sb["amask"][:])\
                .then_inc(dve_sem)                                   # 2 W
            vector.wait_ge(pe_sem, 1)
            nc.vector.tensor_copy(sb["WT"][:], p_wt[:]).then_inc(dve_sem)
            nc.vector.drain()                                        # 2 WT
            nc.vector.tensor_mul(sb["scr"][:], sb["W"][:], sb["WT"][:])\
                .then_inc(dve_sem)                                   # 3 scr
            nc.vector.drain()
            nc.vector.tensor_reduce(nr_v[:], sb["scr"][:], X, Alu.add,
                                    negate=True).then_inc(dve_sem)   # 4 nr
            vector.wait_ge(pe_sem, 2)
            nc.vector.tensor_copy(sb["W2"][:], p_w2[:]).then_inc(dve_sem)
            vector.wait_ge(pe_sem, 3)
            nc.vector.tensor_copy(sb["W2T"][:], p_w2t[:]).then_inc(dve_sem)
            nc.vector.drain()                                        # 7 W2T
            nc.vector.tensor_scalar(sb["y2t"][:], sb["WT"][:], nr_v[:],
                                    None, Alu.mult).then_inc(dve_sem)
            vector.wait_ge(pe_sem, 7)
            nc.vector.scalar_tensor_tensor(
                sb["denom0"][:], sb["W"][:], nr_v[:], p_den[:], Alu.mult,
                Alu.add)
            nc.vector.tensor_mul(sb["wwt"][:], sb["scr"][:], sb["W"][:])
            nc.vector.drain()                                        # 9
            nc.vector.tensor_add(sb["denom"][:], sb["denom0"][:],
                                 sb["wwt"][:]).then_inc(dve_sem, 3)
            nc.vector.drain()                                        # 10
            nc.vector.tensor_scalar(sb["maskp"][:], sb["denom"][:], 0.0,
                                    None, Alu.is_gt)
            nc.vector.tensor_scalar_add(sb["dsafe"][:], sb["denom"][:],
                                        1e-37)
            nc.vector.drain()                                        # 12
            nc.vector.reciprocal(sb["rec"][:], sb["dsafe"][:])
            vector.wait_ge(gp_sem, 1)
            nc.vector.tensor_mul(sb["dmat"][:], sb["rode"][:], offd_s)
            nc.vector.drain()                                        # 14
            nc.vector.tensor_mul(sb["C0"][:], sb["dmat"][:], sb["rec"][:])\
                .then_inc(dve_sem, 5)
            nc.vector.drain()                                        # 15 C0
            nc.vector.tensor_mul(sb["C"][:], sb["C0"][:], sb["maskp"][:])\
                .then_inc(dve_sem)                                   # 16 C
            vector.wait_ge(pe_sem, 8)
            nc.vector.tensor_copy(sb["CT"][:], p_ct[:]).then_inc(dve_sem)
            nc.vector.drain()                                        # 17 CT
            nc.vector.tensor_mul(sb["scrh"][:], sb["W"][:], sb["C"][:])\
                .then_inc(dve_sem)                                   # 18
            nc.vector.drain()
            nc.vector.tensor_reduce(h_v[:], sb["scrh"][:], X, Alu.add)
            nc.vector.drain()                                        # 19
            vector.wait_ge(pe_sem, 9)
            nc.vector.tensor_scalar(nhg_v[:], h_v[:], p_g[:], -1.0, Alu.add,
                                    Alu.mult).then_inc(dve_sem, 2)
            nc.vector.drain()                                        # 20 nhg
            vector.wait_ge(pe_sem, 10)
            nc.vector.tensor_copy(sb["T1"][:], p_t1[:]).then_inc(dve_sem)
            nc.vector.tensor_scalar(sb["z2t"][:], sb["W"][:], nhg_v[:],
                                    None, Alu.mult)
            nc.vector.tensor_scalar(sb["q2t"][:], sb["CT"][:], nr_v[:],
                                    None, Alu.mult)
            vector.wait_ge(gp_sem, 2)
            nc.vector.tensor_mul(sb["wwC"][:], sb["ww"][:], sb["C"][:])\
                .then_inc(dve_sem, 3)
            nc.vector.tensor_mul(sb["wwtc"][:], sb["scr"][:], sb["C"][:])
            vector.wait_ge(pe_sem, 19)
            nc.vector.scalar_tensor_tensor(
                sb["s1"][:], sb["WT"][:], nhg_v[:], p_acc[:], Alu.mult,
                Alu.add)
            nc.vector.drain()                                        # 26
            nc.vector.scalar_tensor_tensor(
                sb["s2"][:], sb["C"][:], nr_v[:], sb["s1"][:], Alu.mult,
                Alu.add)
            nc.vector.drain()                                        # 27
            nc.vector.scalar_tensor_tensor(
                sb["s3"][:], sb["wwtc"][:], 2.0, sb["s2"][:], Alu.mult,
                Alu.add)
            nc.vector.drain()                                        # 28
            nc.vector.tensor_mul(sb["flows_s"][:], sb["W"][:], sb["s3"][:])\
                .then_inc(dve_sem, 5)
            vector.sem_inc(end_sem, 1)

        @block.sync
        def _(sync):
            sync.dma_start(packed.ap(), t_in[:]).then_inc(dma_sem, 16)
            sync.wait_ge(dve_sem, DV["flows"])
            sync.dma_start(t_out[:], sb["flows_s"][:]).then_inc(dma_sem, 16)
            if os.environ.get("KERNEL_NOWAIT", "0") == "1":
                # Do not hold Sync on out-DMA completion: the walrus epilogue
                # (all-engine barrier + sem-file zeroing) starts ~1.8us sooner
                # and overlaps the in-flight transfer. The epilogue zeroes all
                # sems, so the clears below are redundant in this mode.
                return
            sync.wait_ge(dma_sem, 32)
            # join: by data dependence every other engine retired before the
            # out-DMA completed; clearing the sems here is race-free and
            # makes the NEFF safely re-executable with no all-engine barrier
            sync.wait_ge(end_sem, 4)
            sync.nop()
            if os.environ.get("KERNEL_SIM_NOCLEAR", "0") != "1":
                sync.sem_clear(dma_sem)
                sync.sem_clear(dve_sem)
                sync.sem_clear(pe_sem)
                sync.sem_clear(act_sem)
                sync.sem_clear(gp_sem)
                sync.sem_clear(end_sem)

        block_cm.__exit__(None, None, None)

    # strip the Bass-preamble const-memsets + both all-engine barriers;
    # nothing in this program reads the const tensors, and the counting-sem
    # join above replaces the exit barrier
    drop = {"InstMemset", "InstDrain", "InstEventSemaphore"}
    for blk in nc.m.functions[0].blocks:
        if blk.name == "main" or blk.name.endswith("_end"):
            kept = [i for i in blk.instructions
                    if type(i).__name__ not in drop]
            del blk.instructions[:]
            for i in kept:
                blk.instructions.append(i)

    nc.finalize()
    return nc



def _build_dense_nc_v2(lam):
    """Restructured hand-scheduled dense-flows program (v2).

    Feature flags (env, for HW bisection):
      KERNEL_TTR=1      fused multiply+rowsum via tensor_tensor_reduce
      KERNEL_DBL=1      single [110,220] exp/mask/W ops over time|timeT
      KERNEL_POOLADD=1  eye+W / eye+WT / e2 doubling on Pool (else DVE stt)
      KERNEL_RECIP      "recip" (plain DVE reciprocal, default) | "fast"

    Structure vs v1: WT from exp(-timeT) (no PE transpose round-trip);
    row/col scalings as diag(v) matmul members; BCB = (I+W^T)C(I+W^T)
    collapses four matmuls into two; the only post-p_acc DVE work is two
    ops; no trailing out-DMA wait (the walrus epilogue's sem zeroing
    overlaps the transfer and replaces our cleanup).
    """
    _ensure_repo_on_path()
    from contextlib import ExitStack

    import concourse.bacc as bacc
    import concourse.mybir as mybir

    f32 = mybir.dt.float32
    Alu = mybir.AluOpType
    Act = mybir.ActivationFunctionType
    X = mybir.AxisListType.X

    F_TTR = os.environ.get("KERNEL_TTR", "0") == "1"
    F_DBL = os.environ.get("KERNEL_DBL", "0") == "1"
    F_PADD = os.environ.get("KERNEL_POOLADD", "0") == "1"
    F_FASTR = os.environ.get("KERNEL_RECIP", "recip") == "fast"

    CK = 5 * N + 2  # time | timeT | ode | eye | offd | ones | zeros

    nc = bacc.Bacc(None, target_bir_lowering=False)
    t_in = nc.dram_tensor("packed", [N, CK], f32, kind="ExternalInput")
    t_out = nc.dram_tensor("flows", [N, N], f32, kind="ExternalOutput")

    with ExitStack() as ctx:
        in_sem = ctx.enter_context(nc.semaphore("in_sem"))
        out_sem = ctx.enter_context(nc.semaphore("out_sem"))
        dve_sem = ctx.enter_context(nc.semaphore("dve_sem"))
        pe_sem = ctx.enter_context(nc.semaphore("pe_sem"))
        act_sem = ctx.enter_context(nc.semaphore("act_sem"))
        gp_sem = ctx.enter_context(nc.semaphore("gp_sem"))
        block_cm = nc.Block(no_gpsimd_drain=True)
        block = block_cm.__enter__()

        def sbuf(name, cols=N):
            return ctx.enter_context(nc.sbuf_tensor(name, [N, cols], f32))

        def psum(name, cols=N):
            return ctx.enter_context(nc.psum_tensor(name, [N, cols], f32))

        MM = nc.tensor.matmul
        TT = nc.tensor.transpose

        packed = sbuf("packed_s", CK)
        time_s = packed[:, 0:N]
        timeT_s = packed[:, N:2 * N]
        ode_s = packed[:, 2 * N:3 * N]
        eye_s = packed[:, 3 * N:4 * N]
        offd_s = packed[:, 4 * N:5 * N]
        ones_s = packed[:, 5 * N:5 * N + 1]
        zeros_s = packed[:, 5 * N + 1:5 * N + 2]

        names = ["scr", "diag_nr", "y2tn", "W2s", "W2Ts", "denom", "maskp",
                 "rode", "dmat", "dmm", "rec", "C", "CTs", "scrh",
                 "diag_nhg", "A", "Bs", "wwt", "ww", "wwC", "wwtc", "e2",
                 "t2s", "f0", "s3", "flows_s"]
        sb = {n: sbuf(n) for n in names}
        if F_DBL:
            amask2 = sbuf("amask2", 2 * N)
            wexp2 = sbuf("wexp2", 2 * N)
            Wb = sbuf("Wb", 2 * N)
            W_s = Wb[:, 0:N]
            WT_s = Wb[:, N:2 * N]
        else:
            amask2 = sbuf("amask")
            amaskT = sbuf("amaskT")
            wexp2 = sbuf("wexp")
            wexpT = sbuf("wexpT")
            W_t = sbuf("W")
            WT_t = sbuf("WT")
            W_s = W_t[:]
            WT_s = WT_t[:]
        r_v = sbuf("r_v", 1)
        h_v = sbuf("h_v", 1)
        nhg_v = sbuf("nhg_v", 1)
        scratch = sbuf("scratch")
        p_warm = psum("p_warm")
        N_WARM = int(os.environ.get("KERNEL_WARM", "6"))

        p_w2 = psum("p_w2")
        p_w2t = psum("p_w2t")
        p_den = psum("p_den")
        p_ct = psum("p_ct")
        p_g = psum("p_g", 1)
        p_t2 = psum("p_t2")
        p_acc = psum("p_acc")

        # sem counts at key producers (depend on flags; computed in order)
        dv_names = ["amask"]
        if not F_DBL:
            dv_names += ["amaskT"]
        dv_names += ["W"] if F_DBL else ["W", "WT"]
        dv_names += ["scr"]
        if not F_TTR:
            dv_names += ["nr"]
        dv_names += ["diag_nr", "y2tn"]
        if not F_PADD:
            dv_names += ["A", "Bs"]
        dv_names += ["W2s", "W2Ts", "denom", "maskp", "rec", "C", "scrh"]
        if not F_TTR:
            dv_names += ["h"]
        dv_names += ["CTs", "nhg", "diag_nhg", "t2s", "f0", "flows"]
        DV = {n: i + 1 for i, n in enumerate(dv_names)}
        DV["WT"] = DV["W"] if F_DBL else DV["WT"]

        gp_names = ["preload", "dmat"]
        if F_PADD:
            gp_names += ["A", "Bs"]
        gp_names += ["wwt", "ww", "dmm", "wwC", "wwtc"]
        if F_PADD:
            gp_names += ["e2"]
        GP = {n: i + 1 for i, n in enumerate(gp_names)}

        PE = dict(w2=1, w2t=2, den=3, ct=4, g=5, t2=6, acc=7)
        ACTC = dict(wexp=1, wexpT=1 if F_DBL else 2)
        ACTC["rode"] = ACTC["wexpT"] + 1

        @block.scalar
        def _(scalar):
            scalar.wait_ge(in_sem, 16)
            if F_DBL:
                nc.scalar.activation(wexp2[:], packed[:, 0:2 * N], Act.Exp,
                                     bias=zeros_s, scale=-lam)\
                    .then_inc(act_sem)
            else:
                nc.scalar.activation(wexp2[:], time_s, Act.Exp,
                                     bias=zeros_s, scale=-lam)\
                    .then_inc(act_sem)
                nc.scalar.activation(wexpT[:], timeT_s, Act.Exp,
                                     bias=zeros_s, scale=-lam)\
                    .then_inc(act_sem)
            nc.scalar.activation(sb["rode"][:], ode_s, Act.Relu,
                                 bias=zeros_s, scale=1.0).then_inc(act_sem)

        @block.gpsimd
        def _(gpsimd):
            # pre-wait dummy: pulls the Q7 Multiply-library load DMA into
            # the free NEFF-startup window (it otherwise lands mid-kernel
            # and its ~27KB SBUF write stalls every engine ~2us)
            nc.gpsimd.memset(scratch[:, 0:1], 0.0).then_inc(gp_sem)
            gpsimd.wait_ge(gp_sem, 1)
            nc.gpsimd.tensor_mul(scratch[:, 0:1], scratch[:, 0:1],
                                 scratch[:, 0:1])
            gpsimd.wait_ge(in_sem, 16)
            gpsimd.wait_ge(act_sem, ACTC["rode"])
            nc.gpsimd.tensor_mul(sb["dmat"][:], sb["rode"][:], offd_s)\
                .then_inc(gp_sem)
            if F_PADD:
                gpsimd.wait_ge(dve_sem, DV["W"])
                nc.gpsimd.tensor_add(sb["A"][:], eye_s, W_s)\
                    .then_inc(gp_sem)
                gpsimd.wait_ge(dve_sem, DV["WT"])
                nc.gpsimd.tensor_add(sb["Bs"][:], eye_s, WT_s)\
                    .then_inc(gp_sem)
            gpsimd.wait_ge(dve_sem, DV["scr"])
            nc.gpsimd.tensor_mul(sb["wwt"][:], sb["scr"][:], W_s)\
                .then_inc(gp_sem)
            nc.gpsimd.tensor_mul(sb["ww"][:], W_s, W_s).then_inc(gp_sem)
            gpsimd.wait_ge(dve_sem, DV["maskp"])
            gpsimd.wait_ge(gp_sem, GP["dmat"])
            nc.gpsimd.tensor_mul(sb["dmm"][:], sb["dmat"][:],
                                 sb["maskp"][:]).then_inc(gp_sem)
            gpsimd.wait_ge(dve_sem, DV["C"])
            gpsimd.wait_ge(gp_sem, GP["ww"])
            nc.gpsimd.tensor_mul(sb["wwC"][:], sb["ww"][:], sb["C"][:])\
                .then_inc(gp_sem)
            nc.gpsimd.tensor_mul(sb["wwtc"][:], sb["scr"][:], sb["C"][:])\
                .then_inc(gp_sem)
            if F_PADD:
                gpsimd.wait_ge(gp_sem, GP["wwtc"])
                nc.gpsimd.tensor_add(sb["e2"][:], sb["wwtc"][:],
                                     sb["wwtc"][:]).then_inc(gp_sem)

        @block.tensor
        def _(tensor):
            tensor.wait_ge(in_sem, 16)
            for _w in range(N_WARM):
                # p-state warmers: keep PE busy through the DVE head phase
                # so the real matmuls run at ramped clock
                MM(p_warm[:], eye_s, eye_s, start=True, stop=True)
            tensor.wait_ge(dve_sem, DV["WT"])
            MM(p_w2[:], WT_s, W_s, start=True, stop=True)\
                .then_inc(pe_sem)                                    # pe 1
            MM(p_w2t[:], W_s, WT_s, start=True, stop=True)\
                .then_inc(pe_sem)                                    # pe 2
            # p_den = W + W2 + diag(-r)W + W3 + (W^T.*-r)^T
            MM(p_den[:], eye_s, W_s, start=True, stop=False)
            MM(p_den[:], WT_s, W_s, start=False, stop=False)
            tensor.wait_ge(dve_sem, DV["diag_nr"])
            MM(p_den[:], sb["diag_nr"][:], W_s, start=False, stop=False)
            tensor.wait_ge(dve_sem, DV["W2s"])
            MM(p_den[:], WT_s, sb["W2s"][:], start=False, stop=False)
            MM(p_den[:], sb["y2tn"][:], eye_s, is_transpose=True,
               start=False, stop=True).then_inc(pe_sem)              # pe 3
            tensor.wait_ge(dve_sem, DV["C"])
            TT(p_ct[:], sb["C"][:], eye_s).then_inc(pe_sem)          # pe 4
            tensor.wait_ge(dve_sem, DV["scrh"])
            MM(p_g[:], sb["scrh"][:], ones_s, start=True, stop=True)\
                .then_inc(pe_sem)                                    # pe 5
            tensor.wait_ge(dve_sem, DV["CTs"])
            tensor.wait_ge(dve_sem if not F_PADD else gp_sem,
                           (DV if not F_PADD else GP)["Bs"])
            MM(p_t2[:], sb["CTs"][:], sb["Bs"][:], start=True, stop=True)\
                .then_inc(pe_sem)                                    # pe 6
            # p_acc = (W2)^T C + diag(-r)C + C diag(-r) + C(W2)^T
            #         + (W.*W.*C)^T + diag(-hg)W^T + W^T diag(-hg) + BCB
            MM(p_acc[:], sb["W2s"][:], sb["C"][:], start=True, stop=False)
            MM(p_acc[:], sb["diag_nr"][:], sb["C"][:], start=False,
               stop=False)
            MM(p_acc[:], sb["CTs"][:], sb["diag_nr"][:], start=False,
               stop=False)
            MM(p_acc[:], sb["CTs"][:], sb["W2Ts"][:], start=False,
               stop=False)
            tensor.wait_ge(gp_sem, GP["wwC"])
            MM(p_acc[:], sb["wwC"][:], eye_s, is_transpose=True,
               start=False, stop=False)
            tensor.wait_ge(dve_sem, DV["diag_nhg"])
            MM(p_acc[:], sb["diag_nhg"][:], WT_s, start=False, stop=False)
            MM(p_acc[:], W_s, sb["diag_nhg"][:], start=False, stop=False)
            tensor.wait_ge(dve_sem, DV["t2s"])
            tensor.wait_ge(dve_sem if not F_PADD else gp_sem,
                           (DV if not F_PADD else GP)["A"])
            MM(p_acc[:], sb["A"][:], sb["t2s"][:], start=False, stop=True)\
                .then_inc(pe_sem)                                    # pe 7

        @block.vector
        def _(vector):
            vector.wait_ge(in_sem, 16)
            if F_DBL:
                nc.vector.tensor_scalar(amask2[:], packed[:, 0:2 * N], 0.0,
                                        None, Alu.is_gt).then_inc(dve_sem)
                nc.vector.drain()
                vector.wait_ge(act_sem, ACTC["wexp"])
                nc.vector.tensor_mul(Wb[:], wexp2[:], amask2[:])\
                    .then_inc(dve_sem)
                nc.vector.drain()
            else:
                nc.vector.tensor_scalar(amask2[:], time_s, 0.0, None,
                                        Alu.is_gt).then_inc(dve_sem)
                nc.vector.tensor_scalar(amaskT[:], timeT_s, 0.0, None,
                                        Alu.is_gt).then_inc(dve_sem)
                nc.vector.drain()
                vector.wait_ge(act_sem, ACTC["wexp"])
                nc.vector.tensor_mul(W_t[:], wexp2[:], amask2[:])\
                    .then_inc(dve_sem)
                vector.wait_ge(act_sem, ACTC["wexpT"])
                nc.vector.tensor_mul(WT_t[:], wexpT[:], amaskT[:])\
                    .then_inc(dve_sem)
                nc.vector.drain()
            if F_TTR:
                nc.vector.tensor_tensor_reduce(
                    sb["scr"][:], W_s, WT_s, 1.0, 0.0, Alu.mult, Alu.add,
                    r_v[:]).then_inc(dve_sem)
                nc.vector.drain()
                nc.vector.tensor_scalar(sb["diag_nr"][:], eye_s, r_v[:],
                                        -1.0, Alu.mult, Alu.mult)\
                    .then_inc(dve_sem)
                nc.vector.tensor_scalar(sb["y2tn"][:], WT_s, r_v[:], -1.0,
                                        Alu.mult, Alu.mult).then_inc(dve_sem)
            else:
                nc.vector.tensor_mul(sb["scr"][:], W_s, WT_s)\
                    .then_inc(dve_sem)
                nc.vector.drain()
                nc.vector.tensor_reduce(r_v[:], sb["scr"][:], X, Alu.add,
                                        negate=True).then_inc(dve_sem)
                nc.vector.drain()
                nc.vector.tensor_scalar(sb["diag_nr"][:], eye_s, r_v[:],
                                        None, Alu.mult).then_inc(dve_sem)
                nc.vector.tensor_scalar(sb["y2tn"][:], WT_s, r_v[:], None,
                                        Alu.mult).then_inc(dve_sem)
            if not F_PADD:
                nc.vector.tensor_add(sb["A"][:], eye_s, W_s)\
                    .then_inc(dve_sem)
                nc.vector.tensor_add(sb["Bs"][:], eye_s, WT_s)\
                    .then_inc(dve_sem)
            vector.wait_ge(pe_sem, PE["w2"])
            nc.vector.tensor_copy(sb["W2s"][:], p_w2[:]).then_inc(dve_sem)
            vector.wait_ge(pe_sem, PE["w2t"])
            nc.vector.tensor_copy(sb["W2Ts"][:], p_w2t[:]).then_inc(dve_sem)
            vector.wait_ge(pe_sem, PE["den"])
            vector.wait_ge(gp_sem, GP["wwt"])
            # denom + 1e-37 fused: no-path entries end exactly at 1e-37
            nc.vector.scalar_tensor_tensor(
                sb["denom"][:], sb["wwt"][:], 1e-37, p_den[:], Alu.add,
                Alu.add).then_inc(dve_sem)
            nc.vector.drain()
            nc.vector.tensor_scalar(sb["maskp"][:], sb["denom"][:], 1e-20,
                                    None, Alu.is_gt).then_inc(dve_sem)
            if F_FASTR:
                nc.vector.reciprocal_approx_fast(sb["rec"][:],
                                                 sb["denom"][:])\
                    .then_inc(dve_sem)
            else:
                nc.vector.reciprocal(sb["rec"][:], sb["denom"][:])\
                    .then_inc(dve_sem)
            nc.vector.drain()
            vector.wait_ge(gp_sem, GP["dmm"])
            nc.vector.tensor_mul(sb["C"][:], sb["dmm"][:], sb["rec"][:])\
                .then_inc(dve_sem)
            nc.vector.drain()
            if F_TTR:
                nc.vector.tensor_tensor_reduce(
                    sb["scrh"][:], W_s, sb["C"][:], 1.0, 0.0, Alu.mult,
                    Alu.add, h_v[:]).then_inc(dve_sem)
            else:
                nc.vector.tensor_mul(sb["scrh"][:], W_s, sb["C"][:])\
                    .then_inc(dve_sem)
                nc.vector.drain()
                nc.vector.tensor_reduce(h_v[:], sb["scrh"][:], X, Alu.add)\
                    .then_inc(dve_sem)
            vector.wait_ge(pe_sem, PE["ct"])
            nc.vector.tensor_copy(sb["CTs"][:], p_ct[:]).then_inc(dve_sem)
            nc.vector.drain()
            vector.wait_ge(pe_sem, PE["g"])
            nc.vector.tensor_scalar(nhg_v[:], h_v[:], p_g[:], -1.0, Alu.add,
                                    Alu.mult).then_inc(dve_sem)
            nc.vector.drain()
            nc.vector.tensor_scalar(sb["diag_nhg"][:], eye_s, nhg_v[:],
                                    None, Alu.mult).then_inc(dve_sem)
            vector.wait_ge(pe_sem, PE["t2"])
            nc.vector.tensor_copy(sb["t2s"][:], p_t2[:]).then_inc(dve_sem)
            vector.wait_ge(pe_sem, PE["acc"])
            if F_PADD:
                nc.vector.tensor_mul(sb["f0"][:], W_s, p_acc[:])\
                    .then_inc(dve_sem)
                nc.vector.drain()
                vector.wait_ge(gp_sem, GP["e2"])
                nc.vector.tensor_mul(sb["s3"][:], W_s, sb["e2"][:])
                nc.vector.drain()
                nc.vector.tensor_add(sb["flows_s"][:], sb["f0"][:],
                                     sb["s3"][:]).then_inc(dve_sem)
            else:
                vector.wait_ge(gp_sem, GP["wwtc"])
                nc.vector.scalar_tensor_tensor(
                    sb["s3"][:], sb["wwtc"][:], 2.0, p_acc[:], Alu.mult,
                    Alu.add).then_inc(dve_sem)
                nc.vector.drain()
                nc.vector.tensor_mul(sb["flows_s"][:], W_s, sb["s3"][:])\
                    .then_inc(dve_sem)

        @block.sync
        def _(sync):
            sync.dma_start(packed.ap(), t_in[:]).then_inc(in_sem, 16)
            sync.wait_ge(dve_sem, DV["flows"])
            # out-DMA completion sem is required by codegen but unwaited:
            # the walrus epilogue overlaps the in-flight transfer and its
            # sem-file zeroing replaces any cleanup of ours
            sync.dma_start(t_out[:], sb["flows_s"][:]).then_inc(out_sem, 16)
            if os.environ.get("KERNEL_V2WAIT", "0") == "1":
                sync.wait_ge(out_sem, 16)

        block_cm.__exit__(None, None, None)

    drop = {"InstMemset", "InstDrain", "InstEventSemaphore"}
    for blk in nc.m.functions[0].blocks:
        if blk.name == "main" or blk.name.endswith("_end"):
            kept = [i for i in blk.instructions
                    if type(i).__name__ not in drop]
            del blk.instructions[:]
            for i in kept:
                blk.instructions.append(i)

    nc.finalize()
    return nc


def _build_dense_nc(lam):
    """Trace the dense-flows Bass program (compile-time constant lambda).

    Engine split: ACT does the LUT ops (sign/exp/relu/square), PE does all
    matmuls/transposes with maximal PSUM accumulation (incl. accumulating
    transposes), DVE does the remaining elementwise stream (~28 ops).
    """
    _ensure_repo_on_path()
    import concourse.bacc as bacc
    import concourse.mybir as mybir
    import concourse.tile as tile

    f32 = mybir.dt.float32
    Alu = mybir.AluOpType
    Act = mybir.ActivationFunctionType
    X = mybir.AxisListType.X

    nc = bacc.Bacc(None, target_bir_lowering=False)
    # packed input: [:, 0:N]=time_mat, [:, N:2N]=ode, [:, 2N:3N]=eye,
    # [:, 3N:4N]=offdiag, [:, 4N]=ones — one DMA (one sync wait) loads all
    t_in = nc.dram_tensor("packed", [N, 4 * N + 1], f32, kind="ExternalInput")
    t_out = nc.dram_tensor("flows", [N, N], f32, kind="ExternalOutput")

    with tile.TileContext(nc) as tc:
        with (
            tc.tile_pool(name="sbuf", bufs=1) as pool,
            tc.tile_pool(name="psum", bufs=1, space="PSUM") as psum,
        ):
            def sb(tag):
                return pool.tile([N, N], f32, name=tag, tag=tag)

            def ps(tag, shared=True):
                # 8 PSUM banks: transient matmul/transpose outputs rotate
                # through 3 shared slots; accumulation groups get their own
                t = "pp" if shared else tag
                return psum.tile([N, N], f32, name=tag, tag=t,
                                 bufs=(3 if shared else 1))

            packed_s = pool.tile([N, 4 * N + 1], f32, name="packed",
                                 tag="packed")
            nc.sync.dma_start(packed_s[:], t_in[:])
            time_s = packed_s[:, 0:N]
            ode_s = packed_s[:, N:2 * N]
            eye_s = packed_s[:, 2 * N:3 * N]
            offd_s = packed_s[:, 3 * N:4 * N]
            ones_s = packed_s[:, 4 * N:4 * N + 1]

            # W = exp(-lam*time) .* (time > 0)
            amask = sb("amask")
            nc.scalar.activation(amask[:], time_s[:], Act.Sign)
            wexp = sb("wexp")
            nc.scalar.activation(wexp[:], time_s[:], Act.Exp, scale=-lam)
            W = sb("W")
            nc.vector.tensor_mul(W[:], wexp[:], amask[:])

            p_wt = ps("p_wt")
            nc.tensor.transpose(p_wt[:], W[:], eye_s[:])
            WT = sb("WT")
            nc.vector.tensor_copy(WT[:], p_wt[:])

            scr = sb("scr")                    # W .* W^T (reused 3x)
            nc.vector.tensor_mul(scr[:], W[:], WT[:])
            nr_v = pool.tile([N, 1], f32, name="nr", tag="nr")
            nc.vector.tensor_reduce(nr_v[:], scr[:], X, Alu.add, negate=True)

            p_w2 = ps("p_w2")
            nc.tensor.matmul(p_w2[:], WT[:], W[:], start=True, stop=True)
            W2 = sb("W2")
            nc.vector.tensor_copy(W2[:], p_w2[:])
            p_w2t = ps("p_w2t")
            nc.tensor.transpose(p_w2t[:], W2[:], eye_s[:])
            W2T = sb("W2T")
            nc.vector.tensor_copy(W2T[:], p_w2t[:])

            # denom = W + W2 + W3 - W.*(r(+)r) + W.*W.*W^T, accumulated on PE:
            # p_den = W + W2 + W3 + (W^T.*(-r))^T  (last term = -W.*r_cols)
            y2t = sb("y2t")
            nc.vector.tensor_scalar(y2t[:], WT[:], nr_v[:], None, Alu.mult)
            p_den = ps("p_den", shared=False)
            nc.tensor.matmul(p_den[:], eye_s[:], W[:], start=True, stop=False)
            nc.tensor.matmul(p_den[:], eye_s[:], W2[:], start=False,
                             stop=False)
            nc.tensor.matmul(p_den[:], W2T[:], W[:], start=False, stop=False)
            nc.tensor.matmul(p_den[:], y2t[:], eye_s[:], is_transpose=True,
                             start=False, stop=True)
            denom0 = sb("denom0")
            nc.vector.scalar_tensor_tensor(
                denom0[:], W[:], nr_v[:], p_den[:], Alu.mult, Alu.add)
            wwt = sb("wwt")                    # scr .* W = W.*W.*W^T
            nc.vector.tensor_mul(wwt[:], scr[:], W[:])
            denom = sb("denom")
            nc.vector.tensor_add(denom[:], denom0[:], wwt[:])

            # C = D / denom (0 where denom == 0)
            maskp = sb("maskp")
            nc.scalar.activation(maskp[:], denom[:], Act.Sign)
            dsafe = sb("dsafe")
            nc.vector.tensor_scalar_add(dsafe[:], denom[:], 1e-37)
            rec = sb("rec")
            nc.vector.reciprocal(rec[:], dsafe[:])
            rode = sb("rode")
            nc.scalar.activation(rode[:], ode_s[:], Act.Relu)
            dmat = sb("dmat")
            nc.vector.tensor_mul(dmat[:], rode[:], offd_s[:])
            C0 = sb("C0")
            nc.vector.tensor_mul(C0[:], dmat[:], rec[:])
            C = sb("C")
            nc.vector.tensor_mul(C[:], C0[:], maskp[:])

            p_ct = ps("p_ct")
            nc.tensor.transpose(p_ct[:], C[:], eye_s[:])
            CT = sb("CT")
            nc.vector.tensor_copy(CT[:], p_ct[:])

            # h = rowsum(W.*C); g = colsum(W.*C) via PE; nhg = -(h+g)
            scrh = sb("scrh")
            nc.vector.tensor_mul(scrh[:], W[:], C[:])
            h_v = pool.tile([N, 1], f32, name="h", tag="h")
            nc.vector.tensor_reduce(h_v[:], scrh[:], X, Alu.add)
            p_g = psum.tile([N, 1], f32, name="p_g", tag="p_g", bufs=1)
            nc.tensor.matmul(p_g[:], scrh[:], ones_s[:], start=True,
                             stop=True)
            nhg_v = pool.tile([N, 1], f32, name="nhg", tag="nhg")
            nc.vector.tensor_scalar(nhg_v[:], h_v[:], p_g[:], -1.0, Alu.add,
                                    Alu.mult)

            # T1 = C @ W^T (standalone: rhs of T4)
            p_t1 = ps("p_t1")
            nc.tensor.matmul(p_t1[:], CT[:], WT[:], start=True, stop=True)
            T1 = sb("T1")
            nc.vector.tensor_copy(T1[:], p_t1[:])

            ww = sb("ww")
            nc.scalar.activation(ww[:], W[:], Act.Square)
            z2t = sb("z2t")
            nc.vector.tensor_scalar(z2t[:], W[:], nhg_v[:], None, Alu.mult)
            q2t = sb("q2t")
            nc.vector.tensor_scalar(q2t[:], CT[:], nr_v[:], None, Alu.mult)
            wwC = sb("wwC")
            nc.vector.tensor_mul(wwC[:], ww[:], C[:])

            # p_acc = T2 + T3 + T5 + T4 + C + T1
            #         + (-W.*hg)^T + (-C^T.*r)^T + (W.*W.*C)^T
            p_acc = ps("p_acc", shared=False)
            nc.tensor.matmul(p_acc[:], W[:], C[:], start=True, stop=False)
            nc.tensor.matmul(p_acc[:], CT[:], W2T[:], start=False, stop=False)
            nc.tensor.matmul(p_acc[:], W2[:], C[:], start=False, stop=False)
            nc.tensor.matmul(p_acc[:], W[:], T1[:], start=False, stop=False)
            nc.tensor.matmul(p_acc[:], eye_s[:], C[:], start=False,
                             stop=False)
            nc.tensor.matmul(p_acc[:], eye_s[:], T1[:], start=False,
                             stop=False)
            nc.tensor.matmul(p_acc[:], z2t[:], eye_s[:], is_transpose=True,
                             start=False, stop=False)
            nc.tensor.matmul(p_acc[:], q2t[:], eye_s[:], is_transpose=True,
                             start=False, stop=False)
            nc.tensor.matmul(p_acc[:], wwC[:], eye_s[:], is_transpose=True,
                             start=False, stop=True)

            s1 = sb("s1")
            nc.vector.scalar_tensor_tensor(
                s1[:], WT[:], nhg_v[:], p_acc[:], Alu.mult, Alu.add)
            s2 = sb("s2")
            nc.vector.scalar_tensor_tensor(
                s2[:], C[:], nr_v[:], s1[:], Alu.mult, Alu.add)
            wwtc = sb("wwtc")
            nc.vector.tensor_mul(wwtc[:], scr[:], C[:])
            s3 = sb("s3")
            nc.vector.scalar_tensor_tensor(
                s3[:], wwtc[:], 2.0, s2[:], Alu.mult, Alu.add)
            flows_s = sb("flows")
            nc.vector.tensor_mul(flows_s[:], W[:], s3[:])
            nc.sync.dma_start(t_out[:], flows_s[:])

    nc.finalize()
    return nc


def _install_ntff_shim():
    """Best-effort NTFF profiling hook for trace mode (KERNEL_TRACE=1).

    The agent image's antenv lacks axon_hooks; recreate it and register the
    ctypes profiler from the axon boot script so run_bass_kernel_spmd's
    trace path (neuron-profile on the NTFFs) works.
    """
    import types
    try:
        import antenv
        if "antenv.axon_hooks" not in sys.modules:
            mod = types.ModuleType("antenv.axon_hooks")
            _h = [None]
            mod.set_axon_ntff_profile_hook = lambda h: _h.__setitem__(0, h)
            mod.get_axon_ntff_profile_hook = lambda: _h[0]
            sys.modules["antenv.axon_hooks"] = mod
            antenv.axon_hooks = mod
        if sys.modules["antenv.axon_hooks"].get_axon_ntff_profile_hook() \
                is None:
            if "/root/.axon_site" not in sys.path:
                sys.path.insert(0, "/root/.axon_site")
            from trn_agent_boot.trn_boot import _ntff_profile_via_ctypes
            sys.modules["antenv.axon_hooks"].set_axon_ntff_profile_hook(
                _ntff_profile_via_ctypes("/opt/axon/libaxon_pjrt.so"))
        from concourse import bass_utils
        bass_utils.upload_artifacts = lambda tmpdir: f"file://{tmpdir}"
        return True
    except Exception:
        return False


def _run_dense(ode, time_mat, lam):
    global LAST_EXEC_NS
    _ensure_repo_on_path()
    from concourse.bass_utils import run_bass_kernel_spmd

    impl = os.environ.get("KERNEL_IMPL", "v2")
    key = (float(lam), impl)
    if key not in _NC_CACHE:
        builder = {"v2": _build_dense_nc_v2, "raw": _build_dense_nc_raw,
                   "tile": _build_dense_nc}[impl]
        _NC_CACHE[key] = builder(float(lam))
    nc = _NC_CACHE[key]

    if impl == "v2":
        packed = np.zeros((N, 5 * N + 2), np.float32)
        packed[:, 0:N] = time_mat
        packed[:, N:2 * N] = time_mat.T
        packed[:, 2 * N:3 * N] = ode
        packed[:, 3 * N:4 * N] = np.eye(N, dtype=np.float32)
        packed[:, 4 * N:5 * N] = 1.0 - np.eye(N, dtype=np.float32)
        packed[:, 5 * N] = 1.0
    else:
        packed = np.zeros((N, 4 * N + (2 if impl == "raw" else 1)),
                          np.float32)
        packed[:, 0:N] = time_mat
        packed[:, N:2 * N] = ode
        packed[:, 2 * N:3 * N] = np.eye(N, dtype=np.float32)
        packed[:, 3 * N:4 * N] = 1.0 - np.eye(N, dtype=np.float32)
        packed[:, 4 * N] = 1.0
    in_map = {"packed": packed}
    n_cores = 8
    trace = os.environ.get("KERNEL_TRACE", "0") == "1"
    kwargs = {}
    if trace:
        trace = _install_ntff_shim()
        if trace:
            import tempfile
            kwargs["tmpdir"] = tempfile.mkdtemp(prefix="bass_ntff_")
    res = run_bass_kernel_spmd(
        nc, [dict(in_map) for _ in range(n_cores)], list(range(n_cores)),
        trace=trace, **kwargs)
    LAST_EXEC_NS = res.exec_time_ns
    return np.asarray(res.results[0]["flows"], np.float32)


# --------------------------------------------------------------------------
# Fallback: faithful elementwise computation (non-conforming inputs only)
# --------------------------------------------------------------------------

def _fallback(ode, time_mat, path_u, path_v, edge_mask, od_o, od_d, group,
              lam):
    n = ode.shape[0]
    nseg = n * n + 1
    m = edge_mask.astype(np.float64)
    t_p = (time_mat.astype(np.float64)[path_u, path_v] * m).sum(-1)
    logits = -lam * t_p
    gmax = np.full(nseg, -np.inf)
    np.maximum.at(gmax, group, logits)
    gmax = np.where(np.isfinite(gmax), gmax, 0.0)
    e = np.exp(logits - gmax[group])
    den = np.zeros(nseg)
    np.add.at(den, group, e)
    den_safe = np.where(den > 0, den, 1.0)
    probs = e / den_safe[group]
    demand = ode.astype(np.float64)[od_o, od_d]
    demand = np.where((demand > 0) & (od_o != od_d), demand, 0.0)
    contrib = (demand * probs)[:, None] * m
    flat = (path_u.astype(np.int64) * n + path_v).reshape(-1)
    flows = np.zeros(n * n)
    np.add.at(flows, flat, contrib.reshape(-1))
    return flows.reshape(n, n).astype(time_mat.dtype)


# --------------------------------------------------------------------------


def kernel(ode, time_mat, path_u, path_v, edge_mask, od_o, od_d, group,
           lambda_param):
    ode = np.asarray(ode)
    time_mat = np.asarray(time_mat)
    path_u = np.asarray(path_u)
    path_v = np.asarray(path_v)
    edge_mask = np.asarray(edge_mask)
    od_o = np.asarray(od_o)
    od_d = np.asarray(od_d)
    group = np.asarray(group)
    lam = float(np.asarray(lambda_param))

    if 0.0 <= lam <= 8.0 and _inputs_conform(
            time_mat, path_u, path_v, edge_mask, od_o, od_d, group):
        return _run_dense(ode, time_mat, lam)
    return _fallback(ode, time_mat, path_u, path_v, edge_mask, od_o, od_d,
                     group, lam)

